# revision 1
# baseline (speedup 1.0000x reference)
"""Multi-head causal self-attention (B=4, T=2048, C=768, H=12) on 8 trn2 cores.

Sharding: core c handles batch b = c//2 and head-group hg = c%2 (6 heads each).
Each core computes its QKV projection slice, causal attention for its 6 heads,
and a partial output projection (768x2048, transposed). Host sums the two
partials per batch, transposes back, and adds b_o. No cross-core collectives.

All on-chip compute uses a transposed data layout (feature dim on partitions,
token dim on the free axis) so no per-tile transposes are needed in the
attention inner loop; softmax denominators come from an appended ones-row in
the PV matmul; normalization happens after PV via a gpsimd partition
broadcast of the reciprocal denominator. Matmuls run as float32r (full-rate
fp32 mode on the PE; plain fp32 is 4x slower).

Emission order is engine-aware (engines execute in-order): QKV chunk groups
are interleaved with the head pairs they unblock, and within a head the
scores matmul for k-block kb+1 is emitted before the PV matmuls of k-block
kb so the PE never waits on the exp (ACT) of the current block.
"""

import math
import os

import numpy as np

import concourse.bass as bass
from concourse import bacc
import concourse.mybir as mybir
import concourse.tile as tile
from concourse import bass_utils
from concourse.bass import ts
from concourse.masks import make_identity

F32 = mybir.dt.float32
F32R = mybir.dt.float32r

P = 128
T = 2048          # sequence length
C = 768           # embed dim
CS = C // P       # 6 contraction chunks
HL = 6            # heads per core
HD = 64           # head dim
O = 3 * HL * HD   # 1152 rows of the local W_attn slice (q|k|v)
OB = O // P       # 9
J = HL * HD       # 384 local y-feature dim
JS = J // P       # 3
OUTB = C // P     # 6 output row blocks
TT = T // 512     # 4 column tiles of 512


def _build_bass():
    nc = bacc.Bacc("TRN2", target_bir_lowering=False, debug=False)
    x_d = nc.dram_tensor("x", [T, C], F32, kind="ExternalInput").ap()
    w_d = nc.dram_tensor("w", [O, C], F32, kind="ExternalInput").ap()
    b_d = nc.dram_tensor("b", [O], F32, kind="ExternalInput").ap()
    wo_d = nc.dram_tensor("wo", [C, J], F32, kind="ExternalInput").ap()
    out_d = nc.dram_tensor("out", [C, T], F32, kind="ExternalOutput").ap()

    with tile.TileContext(nc) as tc, nc.allow_low_precision(
        reason="fp32r matmul pipeline; fp32 PSUM accumulation throughout"
    ):
        _emit_kernel(tc, x_d, w_d, b_d, wo_d, out_d)
    nc.compile()
    return nc


def _emit_kernel(tc, x_d, w_d, b_d, wo_d, out_d):
    nc = tc.nc
    scale = 1.0 / math.sqrt(HD)

    x_r = x_d.rearrange("(tb p) c -> p tb c", p=P)      # [128, 16, 768]
    w_r = w_d.rearrange("(ob p) c -> p ob c", p=P)      # [128, 9, 768]
    wo_r = wo_d.rearrange("(ob p) j -> p ob j", p=P)    # [128, 6, 384]
    out_r = out_d.rearrange("(ob p) t -> p ob t", p=P)  # [128, 6, 2048]

    with (
        tc.tile_pool(name="persist", bufs=1) as persist,
        tc.tile_pool(name="stage", bufs=2) as stage,
        tc.tile_pool(name="attn", bufs=2) as attn,
        tc.tile_pool(name="ps512", bufs=2, space="PSUM") as ps512,
        tc.tile_pool(name="ps_s", bufs=2, space="PSUM") as ps_s,
        tc.tile_pool(name="ps_y", bufs=2, space="PSUM") as ps_y,
    ):
        ident = persist.tile([P, P], F32)
        make_identity(nc, ident)
        identr = persist.tile([P, P], F32R)
        nc.vector.tensor_copy(identr, ident)
        ones32 = persist.tile([P, HD], F32)
        nc.vector.memset(ones32, 1.0)
        ones1 = persist.tile([1, HD], F32R)
        nc.vector.tensor_copy(ones1, ones32[0:1, :])
        bsb = persist.tile([P, OB], F32)
        nc.sync.dma_start(bsb, b_d.rearrange("(a p) -> p a", p=P))

        xt = persist.tile([P, CS, T], F32R)      # x^T   48KB/partition
        wt = persist.tile([P, CS, O], F32R)      # W^T   27KB
        wot = persist.tile([P, JS, C], F32R)     # Wo^T   9KB
        qkvT = persist.tile([P, OB, T], F32R)    # qkv^T 72KB
        yT = persist.tile([P, JS, T], F32R)      # y^T   24KB

        def transpose_pack(src_tile, n_blk, dst_fn):
            """PE-transpose n_blk [128,128] column blocks of src_tile into a
            packed PSUM tile, then one ACT copy into dst via dst_fn(psum3d)."""
            pk = ps_s.tile([P, 1024], F32, tag="s")
            for i in range(n_blk):
                nc.tensor.transpose(pk[:, ts(i, P)], src_tile[:, ts(i, P)], ident)
            dst_fn(pk[:, : n_blk * P].rearrange("p (a b) -> p a b", b=P))

        ob_order = [0, 3, 6, 1, 4, 7, 2, 5, 8]

        def emit_w(ob):
            wn = stage.tile([P, C], F32, tag="ld", name="wn", bufs=3)
            nc.sync.dma_start(wn[:, : C // 2], w_r[:, ob, : C // 2])
            nc.sync.dma_start(wn[:, C // 2 :], w_r[:, ob, C // 2 :])
            transpose_pack(
                wn, CS, lambda pk, ob=ob: nc.scalar.copy(wt[:, :, ts(ob, P)], pk)
            )

        def emit_x(tb):
            xn = stage.tile([P, C], F32, tag="ld", name="xn", bufs=3)
            nc.sync.dma_start(xn[:, : C // 2], x_r[:, tb, : C // 2])
            nc.sync.dma_start(xn[:, C // 2 :], x_r[:, tb, C // 2 :])
            transpose_pack(
                xn, CS, lambda pk, tb=tb: nc.vector.tensor_copy(xt[:, :, ts(tb, P)], pk)
            )

        def emit_wo():
            for ob in range(OUTB):
                won = stage.tile([P, C], F32, tag="ld", name="won", bufs=3)[:, :J]
                nc.sync.dma_start(won, wo_r[:, ob, :])
                transpose_pack(
                    won, JS,
                    lambda pk, ob=ob: nc.scalar.copy(wot[:, :, ts(ob, P)], pk),
                )

        def emit_qkv(ob, tts=None):
            # qkv^T[o, t] = sum_c W^T[c, o] x^T[c, t] + b[o]
            for tt in (range(TT) if tts is None else tts):
                pq = ps512.tile([P, 512], F32, tag="mm")
                for cs in range(CS):
                    nc.tensor.matmul(
                        pq,
                        wt[:, cs, ts(ob, P)],
                        xt[:, cs, ts(tt, 512)],
                        start=(cs == 0),
                        stop=(cs == CS - 1),
                    )
                nc.vector.tensor_scalar_add(
                    qkvT[:, ob, ts(tt, 512)], pq, bsb[:, ob : ob + 1]
                )

        def emit_head(hl, mid_cb=None, late_cb=None):
            p0 = (hl % 2) * HD
            qT = qkvT[p0 : p0 + HD, hl // 2, :]       # [64, 2048] Q^T
            kT = qkvT[p0 : p0 + HD, 3 + hl // 2, :]   # [64, 2048] K^T
            vT = qkvT[p0 : p0 + HD, 6 + hl // 2, :]   # [64, 2048] V^T
            idd = identr[p0 : p0 + HD, p0 : p0 + HD]

            # V^T -> V (natural [k, d]) with an appended ones column
            vaug = attn.tile([P, T // P, HD + 1], F32R, tag="vaug", bufs=1)
            nc.vector.tensor_copy(
                vaug[:, :, HD : HD + 1], ones32[:, 0 : T // P, None]
            )
            for g in range(2):
                pk = ps512.tile([P, 512], F32R, tag="mm")
                for i in range(8):
                    nc.tensor.transpose(
                        pk[:, ts(i, HD)], vT[:, ts(g * 8 + i, P)], idd
                    )
                nc.vector.tensor_copy(
                    vaug[:, g * 8 : (g + 1) * 8, 0:HD],
                    pk.rearrange("p (a b) -> p a b", b=HD),
                )

            norm_q = []

            def flush_norms():
                while norm_q:
                    qt, yu, rd_t = norm_q.pop(0)
                    bc = ps512.tile([P, 512], F32, tag="mm", name="bc")
                    nc.tensor.matmul(
                        bc[0:HD], ones1[0:1], rd_t[0:1],
                        start=True, stop=True,
                    )
                    nc.vector.tensor_mul(
                        out=yT[p0 : p0 + HD, hl // 2, ts(qt, 512)],
                        in0=yu[0:HD],
                        in1=bc[0:HD],
                    )

            def emit_pv(kb, att, q0, lq, hf, ya_tiles):
                for qt in (2 * hf, 2 * hf + 1):
                    if kb > 4 * qt + 3:
                        continue
                    c0 = max(0, qt * 512 - q0)
                    c1 = min(lq, (qt + 1) * 512 - q0)
                    o0 = q0 + c0 - qt * 512
                    ya = ya_tiles[qt]
                    nc.tensor.matmul(
                        ya[0 : HD + 1, o0 : o0 + (c1 - c0)],
                        vaug[:, kb, :],
                        att[:, c0:c1],
                        start=(kb == 0),
                        stop=(kb == 4 * qt + 3),
                    )
                    if kb == 4 * qt + 3:
                        # normalization part A: pull y+denom out of PSUM and
                        # take the reciprocal; part B is deferred a k-block
                        # so the PE's broadcast matmul never waits on DVE
                        yu = attn.tile([P, 512], F32, tag="yu")
                        nc.vector.tensor_copy(yu[0 : HD + 1], ya[0 : HD + 1])
                        rd_t = attn.tile([P, 512], F32R, tag="yu", name="rd_t")
                        nc.vector.reciprocal(rd_t[0:1], yu[HD : HD + 1])
                        norm_q.append((qt, yu, rd_t))

            # flat (hf, kb) stage list: the scores/exp of stage i+1 are
            # emitted before the PV matmuls of stage i, across hf boundaries
            stages = [(0, kb) for kb in range(8)] + [(1, kb) for kb in range(16)]
            ya_tiles = {}
            pending = None
            for hf, kb in stages:
                if hf == 1 and kb == 14 and late_cb is not None:
                    late_cb()
                if hf == 1 and kb == 0 and mid_cb is not None:
                    # drain hf0 fully (PV + norms) before the callback reads yT
                    if pending is not None:
                        emit_pv(*pending, ya_tiles)
                        pending = None
                    flush_norms()
                    mid_cb()
                for qt in (2 * hf, 2 * hf + 1):
                    if qt not in ya_tiles:
                        ya_tiles[qt] = ps_y.tile(
                            [P, 512], F32, tag="y", name=f"ya{hl}_{qt}"
                        )
                q0 = max(kb * P, hf * 1024)
                lq = (hf + 1) * 1024 - q0
                # scores^T[k, q] for k-block kb, q in [q0, q0+lq)
                sp = ps_s.tile([P, 1024], F32, tag="s")
                for j in range(0, lq, 512):
                    f = min(512, lq - j)
                    nc.tensor.matmul(
                        sp[:, j : j + f],
                        kT[:, ts(kb, P)],
                        qT[:, q0 + j : q0 + j + f],
                        start=True,
                        stop=True,
                    )
                att = attn.tile([P, 1024], F32R, tag="att", bufs=2)
                nc.scalar.activation(
                    att[:, :lq], sp[:, :lq],
                    mybir.ActivationFunctionType.Exp, scale=scale,
                )
                if kb * P == q0:
                    # diagonal block: zero out k > q entries
                    nc.gpsimd.affine_select(
                        out=att[:, :P], in_=att[:, :P],
                        compare_op=mybir.AluOpType.is_ge,
                        fill=0.0, base=0, channel_multiplier=-1,
                        pattern=[[1, P]],
                    )
                flush_norms()
                if pending is not None:
                    emit_pv(*pending, ya_tiles)
                pending = (kb, att, q0, lq, 0 if q0 + lq <= 1024 else 1)
            emit_pv(*pending, ya_tiles)
            flush_norms()

        # ---- output projection: part^T[o, t] = sum_j Wo^T[j, o] y^T[j, t]
        def emit_outproj(tts):
          for tt in tts:
            for ob in range(OUTB):
                po = ps512.tile([P, 512], F32, tag="mm")
                for js in range(JS):
                    nc.tensor.matmul(
                        po,
                        wot[:, js, ts(ob, P)],
                        yT[:, js, ts(tt, 512)],
                        start=(js == 0),
                        stop=(js == JS - 1),
                    )
                osb = stage.tile([P, C], F32, tag="ld", name="osb", bufs=3)[:, :512]
                nc.vector.tensor_copy(osb, po)
                nc.sync.dma_start(
                    out_r[:, ob, ts(2 * tt, 256)], osb[:, 0:256]
                )
                nc.sync.dma_start(
                    out_r[:, ob, ts(2 * tt + 1, 256)], osb[:, 256:512]
                )
          return

        # ---- loads/transposes and QKV group 0, interleaved at tt granularity
        for ob in ob_order[0:3]:
            emit_w(ob)
        qkv_units = []  # (ob, tt) ready once tt's x-blocks are transposed
        for tb in range(T // P):
            emit_x(tb)
            if tb % 4 == 3:
                qkv_units += [(ob, tb // 4) for ob in ob_order[0:3]]
            # drain at most one unit per x-block once available, rest at end
            if qkv_units and tb >= 3:
                ob, tt = qkv_units.pop(0)
                emit_qkv(ob, tts=[tt])
        for ob, tt in qkv_units:
            emit_qkv(ob, tts=[tt])

        # ---- interleave remaining QKV chunk groups with head pairs
        for g in range(3):
            if g > 0:
                for ob in ob_order[3 * g : 3 * g + 3]:
                    emit_w(ob)
                    emit_qkv(ob)
            emit_head(2 * g)
            if g == 2:
                emit_wo()
                emit_head(
                    2 * g + 1,
                    mid_cb=lambda: emit_outproj([0, 1]),
                    late_cb=lambda: emit_outproj([2]),
                )
            else:
                emit_head(2 * g + 1)


        emit_outproj([3])


_NC_CACHE = None
LAST_RESULTS = None


def _get_nc():
    global _NC_CACHE
    if _NC_CACHE is None:
        _NC_CACHE = _build_bass()
    return _NC_CACHE


def kernel(x, W_attn, b_attn, W_o, b_o):
    global LAST_RESULTS
    x = np.asarray(x, np.float32)
    W_attn = np.asarray(W_attn, np.float32)
    b_attn = np.asarray(b_attn, np.float32)
    W_o = np.asarray(W_o, np.float32)
    b_o = np.asarray(b_o, np.float32)

    B = x.shape[0]
    in_maps = []
    for core in range(8):
        b, hg = divmod(core, 2)
        sl = slice(hg * J, (hg + 1) * J)
        w_l = np.concatenate(
            [W_attn[sl], W_attn[768 + hg * J : 768 + (hg + 1) * J],
             W_attn[1536 + hg * J : 1536 + (hg + 1) * J]], axis=0
        )
        b_l = np.concatenate(
            [b_attn[sl], b_attn[768 + hg * J : 768 + (hg + 1) * J],
             b_attn[1536 + hg * J : 1536 + (hg + 1) * J]], axis=0
        )
        in_maps.append({
            "x": np.ascontiguousarray(x[b]),
            "w": np.ascontiguousarray(w_l),
            "b": np.ascontiguousarray(b_l),
            "wo": np.ascontiguousarray(W_o[:, sl]),
        })

    nc = _get_nc()
    LAST_RESULTS = bass_utils.run_bass_kernel_spmd(
        nc, in_maps, core_ids=list(range(8)),
        trace=bool(int(os.environ.get("KERNEL_TRACE", "0"))),
    )
    parts = [r["out"] for r in LAST_RESULTS.results]

    out = np.empty((B, T, C), np.float32)
    for b in range(B):
        out[b] = (parts[2 * b] + parts[2 * b + 1]).T + b_o
    return out



# revision 8
# speedup vs baseline: 1.3693x; 1.3693x over previous
"""Multi-head causal self-attention (B=4, T=2048, C=768, H=12) on 8 trn2 cores.

Sharding: core c handles batch b = c//2 and head-group hg = c%2 (6 heads each).
Each core computes its QKV projection slice, causal attention for its 6 heads,
and a partial output projection (768x2048, transposed). Host sums the two
partials per batch, transposes back, and adds b_o. No cross-core collectives.

Key speed structure vs the fp32r baseline:
- all inputs are pre-transposed (and pre-quantized to fp8e4m3 where used as
  fp8) on the HOST, so the kernel does zero on-chip input transposes;
- QKV projection and the PV matmul run as fp8 DoubleRow matmuls (2 k-tiles
  of 128 contracted per pass at 0.5 cycles/row) except where softmax rows
  have too few summands to average out fp8 noise: rows q < 512 (and the
  t < 512 slice of QKV) stay fp32r, keeping rel err ~3e-3;
- V is produced in natural [t, d] layout directly by the projection (no V
  transposes); softmax denominators come from an appended ones column;
- exp on the ACT engine writes fp8 att tiles already in the DoubleRow
  [128, 2, cols] rhs layout; causal masking is applied pre-exp in PSUM by
  gpsimd affine_select with a -1e5 fill;
- the attention stream is ordered qt-major (q-chunk of 512 across all heads)
  so output-projection chunks of earlier qt overlap later attention instead
  of forming a serial tail; QKV chunk tt feeds attention block qt=tt, which
  only needs K/V up to (qt+1)*512 (causality).
"""

import math
import os

import numpy as np
import ml_dtypes

import concourse.bass as bass
from concourse import bacc
import concourse.mybir as mybir
import concourse.tile as tile
from concourse import bass_utils
from concourse.bass import ts

F32 = mybir.dt.float32
F32R = mybir.dt.float32r
F8 = mybir.dt.float8e4
DR = mybir.MatmulPerfMode.DoubleRow

P = 128
T = 2048          # sequence length
C = 768           # embed dim
CS = C // P       # 6 contraction chunks
HL = 6            # heads per core
HD = 64           # head dim
J = HL * HD       # 384 local y-feature dim
JS = J // P       # 3
O = 3 * J         # 1152 rows of the local W_attn slice (q|k|v)
OB = O // P       # 9
QKOB = 6          # q,k row blocks
OUTB = C // P     # 6 output row blocks
NQT = 4           # 512-col q chunks
NPAIR = 8         # 256-row k pairs
HDP = 72          # padded head stride in vaug (dual-fp8 needs 16B-aligned steps)
SCALE = 1.0 / math.sqrt(HD)


def _build_bass():
    nc = bacc.Bacc("TRN2", target_bir_lowering=False, debug=False)
    xt32_d = nc.dram_tensor("xt32", [C, 512], F32R, kind="ExternalInput").ap()
    xt8_d = nc.dram_tensor("xt8", [C, T], F8, kind="ExternalInput").ap()
    wt32_d = nc.dram_tensor("wt32", [C, O], F32R, kind="ExternalInput").ap()
    wt8_d = nc.dram_tensor("wt8", [C, O], F8, kind="ExternalInput").ap()
    wot_d = nc.dram_tensor("wot", [J, C], F32R, kind="ExternalInput").ap()
    bqk_d = nc.dram_tensor("bqk", [2 * J], F32, kind="ExternalInput").ap()
    bvf_d = nc.dram_tensor("bvf", [P, J], F32, kind="ExternalInput").ap()
    out_d = nc.dram_tensor("out", [C, T], F32, kind="ExternalOutput").ap()

    with tile.TileContext(nc) as tc, nc.allow_low_precision(
        reason="fp8 doublerow + fp32r pipeline; fp32 PSUM accumulation"
    ):
        _emit_kernel(tc, xt32_d, xt8_d, wt32_d, wt8_d, wot_d, bqk_d, bvf_d, out_d)
    nc.compile()
    return nc


def _emit_kernel(tc, xt32_d, xt8_d, wt32_d, wt8_d, wot_d, bqk_d, bvf_d, out_d):
    nc = tc.nc

    xt32_r = xt32_d.rearrange("(cs p) t -> p cs t", p=P)   # [128, 6, 512]
    xt8_r = xt8_d.rearrange("(cs p) t -> p cs t", p=P)     # [128, 6, 2048]
    wt32_r = wt32_d.rearrange("(cs p) o -> p cs o", p=P)   # [128, 6, 1152]
    wt8_r = wt8_d.rearrange("(cs p) o -> p cs o", p=P)     # [128, 6, 1152]
    wot_r = wot_d.rearrange("(jb p) o -> p jb o", p=P)     # [128, 3, 768]
    bqk_r = bqk_d.rearrange("(a p) -> p a", p=P)           # [128, 6]
    out_r = out_d.rearrange("(ob p) t -> p ob t", p=P)     # [128, 6, 2048]

    with (
        tc.tile_pool(name="persist", bufs=1) as persist,
        tc.tile_pool(name="att", bufs=3) as attp,
        tc.tile_pool(name="att32", bufs=2) as attp32,
        tc.tile_pool(name="small", bufs=3) as small,
        tc.tile_pool(name="stage", bufs=3) as stage,
        tc.tile_pool(name="ps_sp", bufs=2, space="PSUM") as ps_sp,
        tc.tile_pool(name="ps_ya", bufs=2, space="PSUM") as ps_ya,
        tc.tile_pool(name="ps_mm", bufs=2, space="PSUM") as ps_mm,
    ):
        # ---- persistent SBUF tensors
        xt32 = persist.tile([P, CS, 512], F32R)     # 12KB/part
        xt8 = persist.tile([P, CS, T], F8)          # 12KB
        wt32 = persist.tile([P, CS, O], F32R)       # 27KB
        wt8 = persist.tile([P, CS, O], F8)          # 6.75KB
        wot = persist.tile([P, JS, C], F32R)        # 9KB
        bqk = persist.tile([P, QKOB], F32)
        bvf = persist.tile([P, J], F32)
        qkT = persist.tile([P, QKOB, T], F32R)      # 48KB  (q ob 0-2, k ob 3-5)
        yT = persist.tile([P, JS, T], F32R)         # 24KB
        vaug8 = persist.tile([P, NPAIR, 2, HL, HDP], F8)      # 6.75KB
        vaug32 = persist.tile([P, 2, 2, HL, HDP], F32R)       # 6.75KB (k<512)
        ones1 = persist.tile([1, HD], F32R)

        onesf = small.tile([P, HD], F32, tag="init", name="onesf")
        nc.vector.memset(onesf, 1.0)
        nc.vector.tensor_copy(ones1, onesf[0:1, :])
        # ones columns of vaug (fp8 1.0 and f32 1.0)
        ones2h = onesf[:, 0 : 2 * HL].rearrange("p (a b) -> p a b", b=HL)
        for pair in range(NPAIR):
            nc.vector.tensor_copy(vaug8[:, pair, :, :, HD], ones2h)
        for pair in range(2):
            nc.vector.tensor_copy(vaug32[:, pair, :, :, HD], ones2h)

        # ---- input DMAs (halved for queue parallelism)
        def dma2(dst, src, axis_len):
            h = axis_len // 2
            nc.sync.dma_start(dst[..., :h], src[..., :h])
            nc.sync.dma_start(dst[..., h:], src[..., h:])

        dma2(wt32, wt32_r, O)
        dma2(xt32, xt32_r, 512)
        nc.sync.dma_start(bqk, bqk_r)
        nc.sync.dma_start(bvf, bvf_d)
        dma2(wt8, wt8_r, O)
        dma2(xt8, xt8_r, T)
        dma2(wot, wot_r, C)

        # ================= building blocks =================

        def emit_qkv32(ob):
            # qkT[:, ob, 0:512] for q/k section ob (0..5), fp32r, t < 512
            pq = ps_mm.tile([P, 512], F32, tag="mm")
            for cs in range(CS):
                nc.tensor.matmul(
                    pq, wt32[:, cs, ts(ob, P)], xt32[:, cs, :],
                    start=(cs == 0), stop=(cs == CS - 1),
                )
            nc.vector.tensor_scalar_add(qkT[:, ob, 0:512], pq, bqk[:, ob : ob + 1])

        def emit_qkv8(ob, tt):
            # qkT[:, ob, tt*512:+512] fp8 DoubleRow, tt in 1..3
            pq = ps_mm.tile([P, 512], F32, tag="mm")
            for i in range(3):
                nc.tensor.matmul(
                    pq,
                    wt8[:, 2 * i : 2 * i + 2, ts(ob, P)],
                    xt8[:, 2 * i : 2 * i + 2, ts(tt, 512)],
                    start=(i == 0), stop=(i == 2), perf_mode=DR,
                )
            nc.vector.tensor_scalar_add(
                qkT[:, ob, ts(tt, 512)], pq, bqk[:, ob : ob + 1]
            )

        def emit_v32(tb):
            # natural-layout v for t-block tb (0..3), fp32r -> vaug32 AND vaug8
            pvt = ps_mm.tile([P, 512], F32, tag="mm", name="pvt")
            pv = pvt[:, 0:J]
            for cs in range(CS):
                nc.tensor.matmul(
                    pv, xt32[:, cs, ts(tb, P)], wt32[:, cs, 2 * J : 3 * J],
                    start=(cs == 0), stop=(cs == CS - 1),
                )
            pair, i = divmod(tb, 2)
            dst32 = vaug32[:, pair, i, :, 0:HD]
            dst8 = vaug8[:, pair, i, :, 0:HD]
            nc.vector.tensor_add(dst32, pv.rearrange("p (h d) -> p h d", d=HD),
                                 bvf.rearrange("p (h d) -> p h d", d=HD))
            nc.vector.tensor_add(dst8, pv.rearrange("p (h d) -> p h d", d=HD),
                                 bvf.rearrange("p (h d) -> p h d", d=HD))

        def emit_v8(tb):
            # natural-layout v for t-block tb (4..15), fp8 DoubleRow -> vaug8
            pvt = ps_mm.tile([P, 512], F32, tag="mm", name="pvt")
            pv = pvt[:, 0:J]
            for i in range(3):
                nc.tensor.matmul(
                    pv,
                    xt8[:, 2 * i : 2 * i + 2, ts(tb, P)],
                    wt8[:, 2 * i : 2 * i + 2, 2 * J : 3 * J],
                    start=(i == 0), stop=(i == 2), perf_mode=DR,
                )
            pair, i = divmod(tb, 2)
            nc.vector.tensor_add(
                vaug8[:, pair, i, :, 0:HD],
                pv.rearrange("p (h d) -> p h d", d=HD),
                bvf.rearrange("p (h d) -> p h d", d=HD),
            )

        def emit_outproj(tt):
            # part^T[o, tt*512:+512] = sum_j wot[j, o] yT[j, t]
            for ob in range(OUTB):
                po = ps_mm.tile([P, 512], F32, tag="mm")
                for js in range(JS):
                    nc.tensor.matmul(
                        po, wot[:, js, ts(ob, P)], yT[:, js, ts(tt, 512)],
                        start=(js == 0), stop=(js == JS - 1),
                    )
                osb = stage.tile([P, 512], F32, tag="osb")
                nc.vector.tensor_copy(osb, po)
                nc.sync.dma_start(out_r[:, ob, ts(2 * tt, 256)], osb[:, 0:256])
                nc.sync.dma_start(out_r[:, ob, ts(2 * tt + 1, 256)], osb[:, 256:512])

        # ================= attention =================
        # unit (h, qt, p): q cols [q0, (qt+1)*512), k pair p (256 rows)

        def unit_geometry(qt, p):
            q0 = max(p * 256, qt * 512)
            cols = (qt + 1) * 512 - q0
            rel = q0 - qt * 512          # 0 or 256
            diag = q0 == p * 256
            return q0, cols, rel, diag

        def emit_scores_exp(h, qt, p):
            """scores (PE) + mask (Pool) + exp (ACT) -> att tile for the unit."""
            q0, cols, rel, diag = unit_geometry(qt, p)
            p0 = (h % 2) * HD
            qTs = qkT[p0 : p0 + HD, h // 2, :]
            kTs = qkT[p0 : p0 + HD, 3 + h // 2, :]
            sp = ps_sp.tile([P, 2, 512], F32, tag="sp")
            # block A (k rows 2p*128..+128): valid from q >= 2p*128 <= q0
            nc.tensor.matmul(
                sp[:, 0, 0:cols], kTs[:, ts(2 * p, P)], qTs[:, q0 : q0 + cols],
                start=True, stop=True,
            )
            # block B: valid from q >= (2p+1)*128; on diagonal units the wedge
            # [0,128) holds finite wrong-side scores, zeroed post-exp below
            nc.tensor.matmul(
                sp[:, 1, 0:cols],
                kTs[:, ts(2 * p + 1, P)], qTs[:, q0 : q0 + cols],
                start=True, stop=True,
            )
            if qt == 0:
                att = attp32.tile([P, 2, 512], F32R, tag="att32")
            else:
                att = attp.tile([P, 2, 512], F8, tag="att")
            nc.scalar.activation(
                att[:, :, 0:cols], sp[:, :, 0:cols],
                mybir.ActivationFunctionType.Exp, scale=SCALE,
            )
            if diag:
                nc.gpsimd.affine_select(
                    out=att[:, 0, 0:P], in_=att[:, 0, 0:P],
                    compare_op=mybir.AluOpType.is_ge,
                    fill=0.0, base=0, channel_multiplier=-1, pattern=[[1, P]],
                )
                nc.gpsimd.affine_select(
                    out=att[:, 1, 0 : 2 * P], in_=att[:, 1, 0 : 2 * P],
                    compare_op=mybir.AluOpType.is_ge,
                    fill=0.0, base=-P, channel_multiplier=-1, pattern=[[1, 2 * P]],
                )
            return att

        def emit_pv(h, qt, p, att, ya):
            q0, cols, rel, diag = unit_geometry(qt, p)
            start = p == 0
            stop = p == 2 * qt + 1
            if qt == 0:
                for i in range(2):
                    nc.tensor.matmul(
                        ya[0 : HD + 1, rel : rel + cols],
                        vaug32[:, p, i, h, 0 : HD + 1],
                        att[:, i, 0:cols],
                        start=(start and i == 0), stop=(stop and i == 1),
                    )
            else:
                nc.tensor.matmul(
                    ya[0 : HD + 1, rel : rel + cols],
                    vaug8[:, p, :, h, 0 : HD + 1],
                    att[:, :, 0:cols],
                    start=start, stop=stop, perf_mode=DR,
                )

        norm_q = []

        def flush_norms():
            while norm_q:
                h, qt, ya = norm_q.pop(0)
                p0 = (h % 2) * HD
                rd = small.tile([1, 512], F32R, tag="rd")
                nc.vector.reciprocal(rd, ya[HD : HD + 1, :])
                bcs = small.tile([HD, 512], F32R, tag="bcs")
                nc.gpsimd.partition_broadcast(bcs, rd)
                nc.vector.tensor_mul(
                    out=yT[p0 : p0 + HD, h // 2, ts(qt, 512)],
                    in0=ya[0:HD], in1=bcs,
                )

        # ================= schedule =================
        fillers = []   # (need_qt, fn): must run before attn block need_qt

        def pump(n=1):
            for _ in range(min(n, len(fillers))):
                fillers.pop(0)[1]()

        def drain(up_to_qt):
            while fillers and fillers[0][0] <= up_to_qt:
                fillers.pop(0)[1]()

        # qt0 prerequisites emitted directly (q,k,v for t<512)
        for ob in (0, 3):
            emit_qkv32(ob)
        emit_v32(0)
        emit_v32(1)
        emit_v32(2)
        emit_v32(3)
        head_ready = [(1, 4), (2, 5)]  # qkv32 obs to emit before heads 2/4

        # fp8 fillers for later qt blocks
        for tt in range(1, NQT):
            for ob in range(QKOB):
                fillers.append((tt, lambda ob=ob, tt=tt: emit_qkv8(ob, tt)))
            for tb in range(4 * tt, 4 * tt + 4):
                fillers.append((tt, lambda tb=tb: emit_v8(tb)))

        pending = None   # (h, qt, p, att, ya)

        for qt in range(NQT):
            drain(qt)
            for h in range(HL):
                if qt == 0 and h in (2, 4):
                    for ob in head_ready[h // 2 - 1]:
                        emit_qkv32(ob)
                ya = ps_ya.tile([P, 512], F32, tag="ya", name=f"ya{h}_{qt}")
                for p in range(2 * qt + 2):
                    att = emit_scores_exp(h, qt, p)
                    flush_norms()
                    if pending is not None:
                        emit_pv(*pending)
                        pending = None
                    pump(1)
                    pending = (h, qt, p, att, ya)
                # close the unit stream for this (h, qt)
                emit_pv(*pending)
                pending = None
                norm_q.append((h, qt, ya))
            # all heads of qt done -> outproj of this qt becomes available
            flush_norms()
            fillers.append((NQT, lambda tt=qt: emit_outproj(tt)))
        drain(NQT)
        flush_norms()


_NC_CACHE = None
LAST_RESULTS = None


def _get_nc():
    global _NC_CACHE
    if _NC_CACHE is None:
        _NC_CACHE = _build_bass()
    return _NC_CACHE


def kernel(x, W_attn, b_attn, W_o, b_o):
    global LAST_RESULTS
    x = np.asarray(x, np.float32)
    W_attn = np.asarray(W_attn, np.float32)
    b_attn = np.asarray(b_attn, np.float32)
    W_o = np.asarray(W_o, np.float32)
    b_o = np.asarray(b_o, np.float32)
    F8NP = ml_dtypes.float8_e4m3

    B = x.shape[0]
    in_maps = []
    for core in range(8):
        b, hg = divmod(core, 2)
        sl = slice(hg * J, (hg + 1) * J)
        w_l = np.concatenate(
            [W_attn[sl], W_attn[C + hg * J : C + (hg + 1) * J],
             W_attn[2 * C + hg * J : 2 * C + (hg + 1) * J]], axis=0
        )  # [1152, 768]
        b_l = np.concatenate(
            [b_attn[sl], b_attn[C + hg * J : C + (hg + 1) * J],
             b_attn[2 * C + hg * J : 2 * C + (hg + 1) * J]], axis=0
        )  # [1152]
        xt = np.ascontiguousarray(x[b].T)              # [768, 2048]
        wt = np.ascontiguousarray(w_l.T)               # [768, 1152]
        in_maps.append({
            "xt32": np.ascontiguousarray(xt[:, :512]),
            "xt8": xt.astype(F8NP),
            "wt32": wt,
            "wt8": wt.astype(F8NP),
            "wot": np.ascontiguousarray(W_o[:, sl].T),  # [384, 768]
            "bqk": np.ascontiguousarray(b_l[: 2 * J]),
            "bvf": np.broadcast_to(b_l[2 * J :], (P, J)).copy(),
        })

    nc = _get_nc()
    LAST_RESULTS = bass_utils.run_bass_kernel_spmd(
        nc, in_maps, core_ids=list(range(8)),
        trace=bool(int(os.environ.get("KERNEL_TRACE", "0"))),
    )
    parts = [r["out"] for r in LAST_RESULTS.results]

    out = np.empty((B, T, C), np.float32)
    for b in range(B):
        out[b] = (parts[2 * b] + parts[2 * b + 1]).T + b_o
    return out


# revision 10
# speedup vs baseline: 1.5246x; 1.1134x over previous
"""Multi-head causal self-attention (B=4, T=2048, C=768, H=12) on 8 trn2 cores.

Sharding: core c handles batch b = c//2 and head-group hg = c%2 (6 heads each).
Each core computes its QKV projection slice, causal attention for its 6 heads,
and a partial output projection (768x2048, transposed). Host sums the two
partials per batch, transposes back, and adds b_o. No cross-core collectives.

Key speed structure vs the fp32r baseline:
- all inputs are pre-transposed (and pre-quantized to fp8e4m3 where used as
  fp8) on the HOST, so the kernel does zero on-chip input transposes;
- QKV projection and the PV matmul run as fp8 DoubleRow matmuls (2 k-tiles
  of 128 contracted per pass at 0.5 cycles/row) except where softmax rows
  have too few summands to average out fp8 noise: rows q < 512 (and the
  t < 512 slice of QKV) stay fp32r, keeping rel err ~3e-3;
- V is produced in natural [t, d] layout directly by the projection (no V
  transposes); softmax denominators come from an appended ones column;
- exp on the ACT engine writes fp8 att tiles already in the DoubleRow
  [128, 2, cols] rhs layout; causal masking is applied pre-exp in PSUM by
  gpsimd affine_select with a -1e5 fill;
- the attention stream is ordered qt-major (q-chunk of 512 across all heads)
  so output-projection chunks of earlier qt overlap later attention instead
  of forming a serial tail; QKV chunk tt feeds attention block qt=tt, which
  only needs K/V up to (qt+1)*512 (causality).
"""

import math
import os

import numpy as np
import ml_dtypes

import concourse.bass as bass
from concourse import bacc
import concourse.mybir as mybir
import concourse.tile as tile
from concourse import bass_utils
from concourse.bass import ts

F32 = mybir.dt.float32
F32R = mybir.dt.float32r
F8 = mybir.dt.float8e4
BF16 = mybir.dt.bfloat16
DR = mybir.MatmulPerfMode.DoubleRow

P = 128
T = 2048          # sequence length
C = 768           # embed dim
CS = C // P       # 6 contraction chunks
HL = 6            # heads per core
HD = 64           # head dim
J = HL * HD       # 384 local y-feature dim
JS = J // P       # 3
O = 3 * J         # 1152 rows of the local W_attn slice (q|k|v)
OB = O // P       # 9
QKOB = 6          # q,k row blocks
OUTB = C // P     # 6 output row blocks
NQT = 4           # 512-col q chunks
NPAIR = 8         # 256-row k pairs
HDP = 72          # padded head stride in vaug (dual-fp8 needs 16B-aligned steps)
SCALE = 1.0 / math.sqrt(HD)


def _build_bass():
    nc = bacc.Bacc("TRN2", target_bir_lowering=False, debug=False)
    xt32_d = nc.dram_tensor("xt32", [C, 512], BF16, kind="ExternalInput").ap()
    xt8_d = nc.dram_tensor("xt8", [C, T], F8, kind="ExternalInput").ap()
    wt32_d = nc.dram_tensor("wt32", [C, O], BF16, kind="ExternalInput").ap()
    wt8_d = nc.dram_tensor("wt8", [C, O], F8, kind="ExternalInput").ap()
    wot_d = nc.dram_tensor("wot", [J, C], F32R, kind="ExternalInput").ap()
    bqk_d = nc.dram_tensor("bqk", [2 * J], F32, kind="ExternalInput").ap()
    bvf_d = nc.dram_tensor("bvf", [P, J], F32, kind="ExternalInput").ap()
    out_d = nc.dram_tensor("out", [C, T], F32, kind="ExternalOutput").ap()

    with tile.TileContext(nc) as tc, nc.allow_low_precision(
        reason="fp8 doublerow + fp32r pipeline; fp32 PSUM accumulation"
    ):
        _emit_kernel(tc, xt32_d, xt8_d, wt32_d, wt8_d, wot_d, bqk_d, bvf_d, out_d)
    nc.compile()
    return nc


def _emit_kernel(tc, xt32_d, xt8_d, wt32_d, wt8_d, wot_d, bqk_d, bvf_d, out_d):
    nc = tc.nc

    xt32_r = xt32_d.rearrange("(cs p) t -> p cs t", p=P)   # [128, 6, 512]
    xt8_r = xt8_d.rearrange("(cs p) t -> p cs t", p=P)     # [128, 6, 2048]
    wt32_r = wt32_d.rearrange("(cs p) o -> p cs o", p=P)   # [128, 6, 1152]
    wt8_r = wt8_d.rearrange("(cs p) o -> p cs o", p=P)     # [128, 6, 1152]
    wot_r = wot_d.rearrange("(jb p) o -> p jb o", p=P)     # [128, 3, 768]
    bqk_r = bqk_d.rearrange("(a p) -> p a", p=P)           # [128, 6]
    out_r = out_d.rearrange("(ob p) t -> p ob t", p=P)     # [128, 6, 2048]

    with (
        tc.tile_pool(name="persist", bufs=1) as persist,
        tc.tile_pool(name="att", bufs=4) as attp,
        tc.tile_pool(name="att32", bufs=3) as attp32,
        tc.tile_pool(name="small", bufs=3) as small,
        tc.tile_pool(name="stage", bufs=3) as stage,
        tc.tile_pool(name="ps_sp", bufs=2, space="PSUM") as ps_sp,
        tc.tile_pool(name="ps_ya", bufs=2, space="PSUM") as ps_ya,
        tc.tile_pool(name="ps_mm", bufs=2, space="PSUM") as ps_mm,
    ):
        # ---- persistent SBUF tensors
        xt32 = persist.tile([P, CS, 512], BF16)     # 6KB/part
        xt8 = persist.tile([P, CS, T], F8)          # 12KB
        wt32 = persist.tile([P, CS, O], BF16)       # 13.5KB
        wt8 = persist.tile([P, CS, O], F8)          # 6.75KB
        wot = persist.tile([P, JS, C], F32R)        # 9KB
        bqk = persist.tile([P, QKOB], F32)
        bvf = persist.tile([P, J], F32)
        qkT = persist.tile([P, QKOB, T], F32R)      # 48KB  (q ob 0-2, k ob 3-5)
        yT = persist.tile([P, JS, T], F32R)         # 24KB
        vaug8 = persist.tile([P, NPAIR, 2, HL, HDP], F8)      # 6.75KB
        vaug32 = persist.tile([P, 2, 2, HL, HDP], F32R)       # 6.75KB (k<512)
        onesf = small.tile([P, HD], F32, tag="init", name="onesf")
        nc.vector.memset(onesf, 1.0)
        # ones columns of vaug (fp8 1.0 and f32 1.0)
        ones2h = onesf[:, 0 : 2 * HL].rearrange("p (a b) -> p a b", b=HL)
        for pair in range(NPAIR):
            nc.vector.tensor_copy(vaug8[:, pair, :, :, HD], ones2h)
        for pair in range(2):
            nc.vector.tensor_copy(vaug32[:, pair, :, :, HD], ones2h)

        # ---- input DMAs (halved for queue parallelism)
        def dma2(dst, src, axis_len):
            h = axis_len // 2
            nc.sync.dma_start(dst[..., :h], src[..., :h])
            nc.sync.dma_start(dst[..., h:], src[..., h:])

        dma2(xt32, xt32_r, 512)
        dma2(wt32, wt32_r, O)
        nc.sync.dma_start(bqk, bqk_r)
        nc.sync.dma_start(bvf, bvf_d)
        dma2(wt8, wt8_r, O)
        dma2(xt8, xt8_r, T)
        dma2(wot, wot_r, C)

        # ================= building blocks =================

        def emit_qkv32(ob):
            # qkT[:, ob, 0:512] for q/k section ob (0..5), fp32r, t < 512
            pq = ps_mm.tile([P, 512], F32, tag="mm")
            for cs in range(CS):
                nc.tensor.matmul(
                    pq, wt32[:, cs, ts(ob, P)], xt32[:, cs, :],
                    start=(cs == 0), stop=(cs == CS - 1),
                )
            nc.vector.tensor_scalar_add(qkT[:, ob, 0:512], pq, bqk[:, ob : ob + 1])

        def emit_qkv8(ob, tt):
            # qkT[:, ob, tt*512:+512] fp8 DoubleRow, tt in 1..3
            pq = ps_mm.tile([P, 512], F32, tag="mm")
            for i in range(3):
                nc.tensor.matmul(
                    pq,
                    wt8[:, 2 * i : 2 * i + 2, ts(ob, P)],
                    xt8[:, 2 * i : 2 * i + 2, ts(tt, 512)],
                    start=(i == 0), stop=(i == 2), perf_mode=DR,
                )
            nc.vector.tensor_scalar_add(
                qkT[:, ob, ts(tt, 512)], pq, bqk[:, ob : ob + 1]
            )

        def emit_v32(tb):
            # natural-layout v for t-block tb (0..3), fp32r -> vaug32 AND vaug8
            pvt = ps_mm.tile([P, 512], F32, tag="mm", name="pvt")
            pv = pvt[:, 0:J]
            for cs in range(CS):
                nc.tensor.matmul(
                    pv, xt32[:, cs, ts(tb, P)], wt32[:, cs, 2 * J : 3 * J],
                    start=(cs == 0), stop=(cs == CS - 1),
                )
            pair, i = divmod(tb, 2)
            dst32 = vaug32[:, pair, i, :, 0:HD]
            dst8 = vaug8[:, pair, i, :, 0:HD]
            nc.vector.tensor_add(dst32, pv.rearrange("p (h d) -> p h d", d=HD),
                                 bvf.rearrange("p (h d) -> p h d", d=HD))
            nc.vector.tensor_add(dst8, pv.rearrange("p (h d) -> p h d", d=HD),
                                 bvf.rearrange("p (h d) -> p h d", d=HD))

        def emit_v8(tb):
            # natural-layout v for t-block tb (4..15), fp8 DoubleRow -> vaug8
            pvt = ps_mm.tile([P, 512], F32, tag="mm", name="pvt")
            pv = pvt[:, 0:J]
            for i in range(3):
                nc.tensor.matmul(
                    pv,
                    xt8[:, 2 * i : 2 * i + 2, ts(tb, P)],
                    wt8[:, 2 * i : 2 * i + 2, 2 * J : 3 * J],
                    start=(i == 0), stop=(i == 2), perf_mode=DR,
                )
            pair, i = divmod(tb, 2)
            nc.vector.tensor_add(
                vaug8[:, pair, i, :, 0:HD],
                pv.rearrange("p (h d) -> p h d", d=HD),
                bvf.rearrange("p (h d) -> p h d", d=HD),
            )

        def emit_outproj(tt):
            # part^T[o, tt*512:+512] = sum_j wot[j, o] yT[j, t]
            for ob in range(OUTB):
                po = ps_mm.tile([P, 512], F32, tag="mm")
                for js in range(JS):
                    nc.tensor.matmul(
                        po, wot[:, js, ts(ob, P)], yT[:, js, ts(tt, 512)],
                        start=(js == 0), stop=(js == JS - 1),
                    )
                osb = stage.tile([P, 512], F32, tag="osb")
                nc.vector.tensor_copy(osb, po)
                nc.sync.dma_start(out_r[:, ob, ts(2 * tt, 256)], osb[:, 0:256])
                nc.sync.dma_start(out_r[:, ob, ts(2 * tt + 1, 256)], osb[:, 256:512])

        # ================= attention =================
        # unit (h, qt, p): q cols [q0, (qt+1)*512), k pair p (256 rows)

        def unit_geometry(qt, p):
            q0 = max(p * 256, qt * 512)
            cols = (qt + 1) * 512 - q0
            rel = q0 - qt * 512          # 0 or 256
            diag = q0 == p * 256
            return q0, cols, rel, diag

        def emit_scores_exp(h, qt, p):
            """scores (PE) + mask (Pool) + exp (ACT) -> att tile for the unit."""
            q0, cols, rel, diag = unit_geometry(qt, p)
            p0 = (h % 2) * HD
            qTs = qkT[p0 : p0 + HD, h // 2, :]
            kTs = qkT[p0 : p0 + HD, 3 + h // 2, :]
            sp = ps_sp.tile([P, 2, 512], F32, tag="sp")
            # block A (k rows 2p*128..+128): valid from q >= 2p*128 <= q0
            nc.tensor.matmul(
                sp[:, 0, 0:cols], kTs[:, ts(2 * p, P)], qTs[:, q0 : q0 + cols],
                start=True, stop=True,
            )
            # block B: valid from q >= (2p+1)*128; on diagonal units the wedge
            # [0,128) holds finite wrong-side scores, zeroed post-exp below
            nc.tensor.matmul(
                sp[:, 1, 0:cols],
                kTs[:, ts(2 * p + 1, P)], qTs[:, q0 : q0 + cols],
                start=True, stop=True,
            )
            if qt == 0:
                att = attp32.tile([P, 2, 512], F32R, tag="att32")
            else:
                att = attp.tile([P, 2, 512], F8, tag="att")
            nc.scalar.activation(
                att[:, :, 0:cols], sp[:, :, 0:cols],
                mybir.ActivationFunctionType.Exp, scale=SCALE,
            )
            if diag:
                nc.gpsimd.affine_select(
                    out=att[:, :, 0 : 2 * P], in_=att[:, :, 0 : 2 * P],
                    compare_op=mybir.AluOpType.is_ge,
                    fill=0.0, base=0, channel_multiplier=-1,
                    pattern=[[-P, 2], [1, 2 * P]],
                )
            return att

        def emit_pv(h, qt, p, att, ya):
            q0, cols, rel, diag = unit_geometry(qt, p)
            start = p == 0
            stop = p == 2 * qt + 1
            if qt == 0:
                for i in range(2):
                    nc.tensor.matmul(
                        ya[0 : HD + 1, rel : rel + cols],
                        vaug32[:, p, i, h, 0 : HD + 1],
                        att[:, i, 0:cols],
                        start=(start and i == 0), stop=(stop and i == 1),
                    )
            else:
                nc.tensor.matmul(
                    ya[0 : HD + 1, rel : rel + cols],
                    vaug8[:, p, :, h, 0 : HD + 1],
                    att[:, :, 0:cols],
                    start=start, stop=stop, perf_mode=DR,
                )

        norm_q = []

        def flush_norms():
            while norm_q:
                h, qt, ya = norm_q.pop(0)
                p0 = (h % 2) * HD
                rd = small.tile([1, 512], F32R, tag="rd")
                nc.vector.reciprocal(rd, ya[HD : HD + 1, :])
                bcs = small.tile([HD, 512], F32R, tag="bcs")
                nc.gpsimd.partition_broadcast(bcs, rd)
                nc.vector.tensor_mul(
                    out=yT[p0 : p0 + HD, h // 2, ts(qt, 512)],
                    in0=ya[0:HD], in1=bcs,
                )

        # ================= schedule =================
        fillers = []   # (need_qt, fn): must run before attn block need_qt

        def pump(n=1):
            for _ in range(min(n, len(fillers))):
                fillers.pop(0)[1]()

        def drain(up_to_qt):
            while fillers and fillers[0][0] <= up_to_qt:
                fillers.pop(0)[1]()

        # qt0 prerequisites emitted directly (q,k,v for t<512)
        for ob in (0, 3):
            emit_qkv32(ob)
        emit_v32(0)
        emit_v32(1)
        emit_v32(2)
        emit_v32(3)
        head_ready = [(1, 4), (2, 5)]  # qkv32 obs to emit before heads 2/4

        # fp8 fillers for later qt blocks
        for tt in range(1, NQT):
            for ob in range(QKOB):
                fillers.append((tt, lambda ob=ob, tt=tt: emit_qkv8(ob, tt)))
            for tb in range(4 * tt, 4 * tt + 4):
                fillers.append((tt, lambda tb=tb: emit_v8(tb)))

        pend = []   # deferred PV units: (h, qt, p, att, ya, last)

        def pop_unit():
            h, qt, p, att, ya, last = pend.pop(0)
            emit_pv(h, qt, p, att, ya)
            if last:
                norm_q.append((h, qt, ya))

        n_attn_units = 0
        for qt in range(NQT):
            drain(qt)
            for h in range(HL):
                if qt == 0 and h in (2, 4):
                    for ob in head_ready[h // 2 - 1]:
                        emit_qkv32(ob)
                ya = ps_ya.tile([P, 512], F32, tag="ya", name=f"ya{h}_{qt}")
                for p in range(2 * qt + 2):
                    flush_norms()
                    att = emit_scores_exp(h, qt, p)
                    pend.append((h, qt, p, att, ya, p == 2 * qt + 1))
                    while len(pend) > 2:
                        pop_unit()
                    n_attn_units += 1
                    # don't pull fp8-dependent fillers into the PE stream
                    # before their DMAs have landed (~early qt0)
                    if n_attn_units > 6:
                        pump(1)
            # all heads of qt done -> outproj of this qt becomes available
            while pend:
                pop_unit()
            flush_norms()
            fillers.append((NQT, lambda tt=qt: emit_outproj(tt)))
        drain(NQT)
        flush_norms()


_NC_CACHE = None
LAST_RESULTS = None


def _get_nc():
    global _NC_CACHE
    if _NC_CACHE is None:
        _NC_CACHE = _build_bass()
    return _NC_CACHE


def kernel(x, W_attn, b_attn, W_o, b_o):
    global LAST_RESULTS
    x = np.asarray(x, np.float32)
    W_attn = np.asarray(W_attn, np.float32)
    b_attn = np.asarray(b_attn, np.float32)
    W_o = np.asarray(W_o, np.float32)
    b_o = np.asarray(b_o, np.float32)
    F8NP = ml_dtypes.float8_e4m3

    B = x.shape[0]
    in_maps = []
    for core in range(8):
        b, hg = divmod(core, 2)
        sl = slice(hg * J, (hg + 1) * J)
        w_l = np.concatenate(
            [W_attn[sl], W_attn[C + hg * J : C + (hg + 1) * J],
             W_attn[2 * C + hg * J : 2 * C + (hg + 1) * J]], axis=0
        )  # [1152, 768]
        b_l = np.concatenate(
            [b_attn[sl], b_attn[C + hg * J : C + (hg + 1) * J],
             b_attn[2 * C + hg * J : 2 * C + (hg + 1) * J]], axis=0
        )  # [1152]
        xt = np.ascontiguousarray(x[b].T)              # [768, 2048]
        wt = np.ascontiguousarray(w_l.T)               # [768, 1152]
        in_maps.append({
            "xt32": np.ascontiguousarray(xt[:, :512]).astype(ml_dtypes.bfloat16),
            "xt8": xt.astype(F8NP),
            "wt32": wt.astype(ml_dtypes.bfloat16),
            "wt8": wt.astype(F8NP),
            "wot": np.ascontiguousarray(W_o[:, sl].T),  # [384, 768]
            "bqk": np.ascontiguousarray(b_l[: 2 * J]),
            "bvf": np.broadcast_to(b_l[2 * J :], (P, J)).copy(),
        })

    nc = _get_nc()
    LAST_RESULTS = bass_utils.run_bass_kernel_spmd(
        nc, in_maps, core_ids=list(range(8)),
        trace=bool(int(os.environ.get("KERNEL_TRACE", "0"))),
    )
    parts = [r["out"] for r in LAST_RESULTS.results]

    out = np.empty((B, T, C), np.float32)
    for b in range(B):
        out[b] = (parts[2 * b] + parts[2 * b + 1]).T + b_o
    return out


# revision 11
# speedup vs baseline: 1.5410x; 1.0108x over previous
"""Multi-head causal self-attention (B=4, T=2048, C=768, H=12) on 8 trn2 cores.

Sharding: core c handles batch b = c//2 and head-group hg = c%2 (6 heads each).
Each core computes its QKV projection slice, causal attention for its 6 heads,
and a partial output projection (768x2048, transposed). Host sums the two
partials per batch, transposes back, and adds b_o. No cross-core collectives.

Key speed structure vs the fp32r baseline:
- all inputs are pre-transposed (and pre-quantized to fp8e4m3 where used as
  fp8) on the HOST, so the kernel does zero on-chip input transposes;
- QKV projection and the PV matmul run as fp8 DoubleRow matmuls (2 k-tiles
  of 128 contracted per pass at 0.5 cycles/row) except where softmax rows
  have too few summands to average out fp8 noise: rows q < 512 (and the
  t < 512 slice of QKV) stay fp32r, keeping rel err ~3e-3;
- V is produced in natural [t, d] layout directly by the projection (no V
  transposes); softmax denominators come from an appended ones column;
- exp on the ACT engine writes fp8 att tiles already in the DoubleRow
  [128, 2, cols] rhs layout; causal masking is applied pre-exp in PSUM by
  gpsimd affine_select with a -1e5 fill;
- the attention stream is ordered qt-major (q-chunk of 512 across all heads)
  so output-projection chunks of earlier qt overlap later attention instead
  of forming a serial tail; QKV chunk tt feeds attention block qt=tt, which
  only needs K/V up to (qt+1)*512 (causality).
"""

import math
import os

import numpy as np
import ml_dtypes

import concourse.bass as bass
from concourse import bacc
import concourse.mybir as mybir
import concourse.tile as tile
from concourse import bass_utils
from concourse.bass import ts

F32 = mybir.dt.float32
F32R = mybir.dt.float32r
F8 = mybir.dt.float8e4
BF16 = mybir.dt.bfloat16
DR = mybir.MatmulPerfMode.DoubleRow

P = 128
T = 2048          # sequence length
C = 768           # embed dim
CS = C // P       # 6 contraction chunks
HL = 6            # heads per core
HD = 64           # head dim
J = HL * HD       # 384 local y-feature dim
JS = J // P       # 3
O = 3 * J         # 1152 rows of the local W_attn slice (q|k|v)
OB = O // P       # 9
QKOB = 6          # q,k row blocks
OUTB = C // P     # 6 output row blocks
NQT = 4           # 512-col q chunks
NPAIR = 8         # 256-row k pairs
HDP = 72          # padded head stride in vaug (dual-fp8 needs 16B-aligned steps)
SCALE = 1.0 / math.sqrt(HD)


def _build_bass():
    nc = bacc.Bacc("TRN2", target_bir_lowering=False, debug=False)
    xt32_d = nc.dram_tensor("xt32", [C, 512], BF16, kind="ExternalInput").ap()
    xt8_d = nc.dram_tensor("xt8", [C, T], F8, kind="ExternalInput").ap()
    wt32_d = nc.dram_tensor("wt32", [C, O], BF16, kind="ExternalInput").ap()
    wt8_d = nc.dram_tensor("wt8", [C, O], F8, kind="ExternalInput").ap()
    wot_d = nc.dram_tensor("wot", [J, C], F32R, kind="ExternalInput").ap()
    bqk_d = nc.dram_tensor("bqk", [2 * J], F32, kind="ExternalInput").ap()
    bvf_d = nc.dram_tensor("bvf", [P, J], F32, kind="ExternalInput").ap()
    out_d = nc.dram_tensor("out", [C, T], F32, kind="ExternalOutput").ap()

    with tile.TileContext(nc) as tc, nc.allow_low_precision(
        reason="fp8 doublerow + fp32r pipeline; fp32 PSUM accumulation"
    ):
        _emit_kernel(tc, xt32_d, xt8_d, wt32_d, wt8_d, wot_d, bqk_d, bvf_d, out_d)
    nc.compile()
    return nc


def _emit_kernel(tc, xt32_d, xt8_d, wt32_d, wt8_d, wot_d, bqk_d, bvf_d, out_d):
    nc = tc.nc

    xt32_r = xt32_d.rearrange("(cs p) t -> p cs t", p=P)   # [128, 6, 512]
    xt8_r = xt8_d.rearrange("(cs p) t -> p cs t", p=P)     # [128, 6, 2048]
    wt32_r = wt32_d.rearrange("(cs p) o -> p cs o", p=P)   # [128, 6, 1152]
    wt8_r = wt8_d.rearrange("(cs p) o -> p cs o", p=P)     # [128, 6, 1152]
    wot_r = wot_d.rearrange("(jb p) o -> p jb o", p=P)     # [128, 3, 768]
    bqk_r = bqk_d.rearrange("(a p) -> p a", p=P)           # [128, 6]
    out_r = out_d.rearrange("(ob p) t -> p ob t", p=P)     # [128, 6, 2048]

    with (
        tc.tile_pool(name="persist", bufs=1) as persist,
        tc.tile_pool(name="att", bufs=4) as attp,
        tc.tile_pool(name="att32", bufs=3) as attp32,
        tc.tile_pool(name="small", bufs=3) as small,
        tc.tile_pool(name="stage", bufs=3) as stage,
        tc.tile_pool(name="oacc", bufs=6) as oaccp,
        tc.tile_pool(name="ps_sp", bufs=2, space="PSUM") as ps_sp,
        tc.tile_pool(name="ps_ya", bufs=2, space="PSUM") as ps_ya,
        tc.tile_pool(name="ps_mm", bufs=2, space="PSUM") as ps_mm,
    ):
        # ---- persistent SBUF tensors
        xt32 = persist.tile([P, CS, 512], BF16)     # 6KB/part
        xt8 = persist.tile([P, CS, T], F8)          # 12KB
        wt32 = persist.tile([P, CS, O], BF16)       # 13.5KB
        wt8 = persist.tile([P, CS, O], F8)          # 6.75KB
        wot = persist.tile([P, JS, C], F32R)        # 9KB
        bqk = persist.tile([P, QKOB], F32)
        bvf = persist.tile([P, J], F32)
        qkT = persist.tile([P, QKOB, T], F32R)      # 48KB  (q ob 0-2, k ob 3-5)
        yT = persist.tile([P, JS, T], F32R)         # 24KB
        vaug8 = persist.tile([P, NPAIR, 2, HL, HDP], F8)      # 6.75KB
        vaug32 = persist.tile([P, 2, 2, HL, HDP], F32R)       # 6.75KB (k<512)
        onesf = small.tile([P, HD], F32, tag="init", name="onesf")
        nc.vector.memset(onesf, 1.0)
        # ones columns of vaug (fp8 1.0 and f32 1.0)
        ones2h = onesf[:, 0 : 2 * HL].rearrange("p (a b) -> p a b", b=HL)
        for pair in range(NPAIR):
            nc.vector.tensor_copy(vaug8[:, pair, :, :, HD], ones2h)
        for pair in range(2):
            nc.vector.tensor_copy(vaug32[:, pair, :, :, HD], ones2h)

        # ---- input DMAs (halved for queue parallelism)
        def dma2(dst, src, axis_len):
            h = axis_len // 2
            nc.sync.dma_start(dst[..., :h], src[..., :h])
            nc.sync.dma_start(dst[..., h:], src[..., h:])

        dma2(xt32, xt32_r, 512)
        nc.sync.dma_start(wt32[:, :, 0:J], wt32_r[:, :, 0:J])
        nc.sync.dma_start(wt32[:, :, J : 2 * J], wt32_r[:, :, J : 2 * J])
        nc.sync.dma_start(wt32[:, :, 2 * J :], wt32_r[:, :, 2 * J :])
        nc.sync.dma_start(bqk, bqk_r)
        nc.sync.dma_start(bvf, bvf_d)
        dma2(wt8, wt8_r, O)
        dma2(xt8, xt8_r, T)
        dma2(wot, wot_r, C)

        # ================= building blocks =================

        def emit_qkv32(ob):
            # qkT[:, ob, 0:512] for q/k section ob (0..5), fp32r, t < 512
            pq = ps_mm.tile([P, 512], F32, tag="mm")
            for cs in range(CS):
                nc.tensor.matmul(
                    pq, wt32[:, cs, ts(ob, P)], xt32[:, cs, :],
                    start=(cs == 0), stop=(cs == CS - 1),
                )
            nc.vector.tensor_scalar_add(qkT[:, ob, 0:512], pq, bqk[:, ob : ob + 1])

        def emit_qkv8(ob, tt):
            # qkT[:, ob, tt*512:+512] fp8 DoubleRow, tt in 1..3
            pq = ps_mm.tile([P, 512], F32, tag="mm")
            for i in range(3):
                nc.tensor.matmul(
                    pq,
                    wt8[:, 2 * i : 2 * i + 2, ts(ob, P)],
                    xt8[:, 2 * i : 2 * i + 2, ts(tt, 512)],
                    start=(i == 0), stop=(i == 2), perf_mode=DR,
                )
            nc.vector.tensor_scalar_add(
                qkT[:, ob, ts(tt, 512)], pq, bqk[:, ob : ob + 1]
            )

        def emit_v32(tb):
            # natural-layout v for t-block tb (0..3), fp32r -> vaug32 AND vaug8
            pvt = ps_mm.tile([P, 512], F32, tag="mm", name="pvt")
            pv = pvt[:, 0:J]
            for cs in range(CS):
                nc.tensor.matmul(
                    pv, xt32[:, cs, ts(tb, P)], wt32[:, cs, 2 * J : 3 * J],
                    start=(cs == 0), stop=(cs == CS - 1),
                )
            pair, i = divmod(tb, 2)
            dst32 = vaug32[:, pair, i, :, 0:HD]
            dst8 = vaug8[:, pair, i, :, 0:HD]
            nc.vector.tensor_add(dst32, pv.rearrange("p (h d) -> p h d", d=HD),
                                 bvf.rearrange("p (h d) -> p h d", d=HD))
            nc.vector.tensor_add(dst8, pv.rearrange("p (h d) -> p h d", d=HD),
                                 bvf.rearrange("p (h d) -> p h d", d=HD))

        def emit_v8(tb):
            # natural-layout v for t-block tb (4..15), fp8 DoubleRow -> vaug8
            pvt = ps_mm.tile([P, 512], F32, tag="mm", name="pvt")
            pv = pvt[:, 0:J]
            for i in range(3):
                nc.tensor.matmul(
                    pv,
                    xt8[:, 2 * i : 2 * i + 2, ts(tb, P)],
                    wt8[:, 2 * i : 2 * i + 2, 2 * J : 3 * J],
                    start=(i == 0), stop=(i == 2), perf_mode=DR,
                )
            pair, i = divmod(tb, 2)
            nc.vector.tensor_add(
                vaug8[:, pair, i, :, 0:HD],
                pv.rearrange("p (h d) -> p h d", d=HD),
                bvf.rearrange("p (h d) -> p h d", d=HD),
            )

        def emit_outproj(tt):
            # part^T[o, tt*512:+512] = sum_j wot[j, o] yT[j, t]
            for ob in range(OUTB):
                po = ps_mm.tile([P, 512], F32, tag="mm")
                for js in range(JS):
                    nc.tensor.matmul(
                        po, wot[:, js, ts(ob, P)], yT[:, js, ts(tt, 512)],
                        start=(js == 0), stop=(js == JS - 1),
                    )
                osb = stage.tile([P, 512], F32, tag="osb")
                nc.vector.tensor_copy(osb, po)
                nc.sync.dma_start(out_r[:, ob, ts(2 * tt, 256)], osb[:, 0:256])
                nc.sync.dma_start(out_r[:, ob, ts(2 * tt + 1, 256)], osb[:, 256:512])

        oacc_tiles = {}

        def emit_outproj_p1(tt, ob):
            # heads 0-3 contribution (jb 0,1) -> SBUF accumulator
            po = ps_mm.tile([P, 512], F32, tag="mm")
            for js in range(JS - 1):
                nc.tensor.matmul(
                    po, wot[:, js, ts(ob, P)], yT[:, js, ts(tt, 512)],
                    start=(js == 0), stop=(js == JS - 2),
                )
            oa = oaccp.tile([P, 512], F32, tag="oacc", name=f"oa{ob}")
            nc.vector.tensor_copy(oa, po)
            oacc_tiles[ob] = oa

        def emit_outproj_p2(tt, ob):
            # heads 4,5 (jb 2) + accumulator -> DRAM
            po = ps_mm.tile([P, 512], F32, tag="mm")
            nc.tensor.matmul(
                po, wot[:, JS - 1, ts(ob, P)], yT[:, JS - 1, ts(tt, 512)],
                start=True, stop=True,
            )
            osb = stage.tile([P, 512], F32, tag="osb")
            nc.vector.tensor_add(osb, po, oacc_tiles[ob])
            nc.sync.dma_start(out_r[:, ob, ts(2 * tt, 256)], osb[:, 0:256])
            nc.sync.dma_start(out_r[:, ob, ts(2 * tt + 1, 256)], osb[:, 256:512])

        # ================= attention =================
        # unit (h, qt, p): q cols [q0, (qt+1)*512), k pair p (256 rows)

        def unit_geometry(qt, p):
            q0 = max(p * 256, qt * 512)
            cols = (qt + 1) * 512 - q0
            rel = q0 - qt * 512          # 0 or 256
            diag = q0 == p * 256
            return q0, cols, rel, diag

        def emit_scores_exp(h, qt, p):
            """scores (PE) + mask (Pool) + exp (ACT) -> att tile for the unit."""
            q0, cols, rel, diag = unit_geometry(qt, p)
            p0 = (h % 2) * HD
            qTs = qkT[p0 : p0 + HD, h // 2, :]
            kTs = qkT[p0 : p0 + HD, 3 + h // 2, :]
            sp = ps_sp.tile([P, 2, 512], F32, tag="sp")
            # block A (k rows 2p*128..+128): valid from q >= 2p*128 <= q0
            nc.tensor.matmul(
                sp[:, 0, 0:cols], kTs[:, ts(2 * p, P)], qTs[:, q0 : q0 + cols],
                start=True, stop=True,
            )
            # block B: valid from q >= (2p+1)*128; on diagonal units the wedge
            # [0,128) holds finite wrong-side scores, zeroed post-exp below
            nc.tensor.matmul(
                sp[:, 1, 0:cols],
                kTs[:, ts(2 * p + 1, P)], qTs[:, q0 : q0 + cols],
                start=True, stop=True,
            )
            if qt == 0:
                att = attp32.tile([P, 2, 512], F32R, tag="att32")
            else:
                att = attp.tile([P, 2, 512], F8, tag="att")
            nc.scalar.activation(
                att[:, :, 0:cols], sp[:, :, 0:cols],
                mybir.ActivationFunctionType.Exp, scale=SCALE,
            )
            if diag:
                nc.gpsimd.affine_select(
                    out=att[:, :, 0 : 2 * P], in_=att[:, :, 0 : 2 * P],
                    compare_op=mybir.AluOpType.is_ge,
                    fill=0.0, base=0, channel_multiplier=-1,
                    pattern=[[-P, 2], [1, 2 * P]],
                )
            return att

        def emit_pv(h, qt, p, att, ya):
            q0, cols, rel, diag = unit_geometry(qt, p)
            start = p == 0
            stop = p == 2 * qt + 1
            if qt == 0:
                for i in range(2):
                    nc.tensor.matmul(
                        ya[0 : HD + 1, rel : rel + cols],
                        vaug32[:, p, i, h, 0 : HD + 1],
                        att[:, i, 0:cols],
                        start=(start and i == 0), stop=(stop and i == 1),
                    )
            else:
                nc.tensor.matmul(
                    ya[0 : HD + 1, rel : rel + cols],
                    vaug8[:, p, :, h, 0 : HD + 1],
                    att[:, :, 0:cols],
                    start=start, stop=stop, perf_mode=DR,
                )

        norm_q = []

        def flush_norms():
            while norm_q:
                h, qt, ya = norm_q.pop(0)
                p0 = (h % 2) * HD
                rd = small.tile([1, 512], F32R, tag="rd")
                nc.vector.reciprocal(rd, ya[HD : HD + 1, :])
                bcs = small.tile([HD, 512], F32R, tag="bcs")
                nc.gpsimd.partition_broadcast(bcs, rd)
                nc.vector.tensor_mul(
                    out=yT[p0 : p0 + HD, h // 2, ts(qt, 512)],
                    in0=ya[0:HD], in1=bcs,
                )

        # ================= schedule =================
        fillers = []   # (need_qt, fn): must run before attn block need_qt

        def pump(n=1):
            for _ in range(min(n, len(fillers))):
                fillers.pop(0)[1]()

        def drain(up_to_qt):
            while fillers and fillers[0][0] <= up_to_qt:
                fillers.pop(0)[1]()

        # qt0 prerequisites emitted directly (q,k,v for t<512)
        for ob in (0, 3):
            emit_qkv32(ob)
        emit_v32(0)
        emit_v32(1)
        emit_v32(2)
        emit_v32(3)
        head_ready = [(1, 4), (2, 5)]  # qkv32 obs to emit before heads 2/4

        # fp8 fillers for later qt blocks
        for tt in range(1, NQT):
            for ob in range(QKOB):
                fillers.append((tt, lambda ob=ob, tt=tt: emit_qkv8(ob, tt)))
            for tb in range(4 * tt, 4 * tt + 4):
                fillers.append((tt, lambda tb=tb: emit_v8(tb)))

        pend = []   # deferred PV units: (h, qt, p, att, ya, last)

        def pop_unit():
            h, qt, p, att, ya, last = pend.pop(0)
            emit_pv(h, qt, p, att, ya)
            if last:
                norm_q.append((h, qt, ya))

        n_attn_units = 0
        for qt in range(NQT):
            drain(qt)
            for h in range(HL):
                if qt == 0 and h in (2, 4):
                    for ob in head_ready[h // 2 - 1]:
                        emit_qkv32(ob)
                if qt == NQT - 1 and h == HL - 1:
                    for ob in range(OUTB):
                        fillers.append((NQT, lambda ob=ob: emit_outproj_p1(3, ob)))
                ya = ps_ya.tile([P, 512], F32, tag="ya", name=f"ya{h}_{qt}")
                for p in range(2 * qt + 2):
                    flush_norms()
                    att = emit_scores_exp(h, qt, p)
                    pend.append((h, qt, p, att, ya, p == 2 * qt + 1))
                    while len(pend) > 2:
                        pop_unit()
                    n_attn_units += 1
                    # don't pull fp8-dependent fillers into the PE stream
                    # before their DMAs have landed (~early qt0)
                    if n_attn_units > 6:
                        pump(1)
            # all heads of qt done -> outproj of this qt becomes available
            while pend:
                pop_unit()
            flush_norms()
            if qt < NQT - 1:
                fillers.append((NQT, lambda tt=qt: emit_outproj(tt)))
        drain(NQT)
        flush_norms()
        for ob in range(OUTB):
            emit_outproj_p2(3, ob)


_NC_CACHE = None
LAST_RESULTS = None


def _get_nc():
    global _NC_CACHE
    if _NC_CACHE is None:
        _NC_CACHE = _build_bass()
    return _NC_CACHE


def kernel(x, W_attn, b_attn, W_o, b_o):
    global LAST_RESULTS
    x = np.asarray(x, np.float32)
    W_attn = np.asarray(W_attn, np.float32)
    b_attn = np.asarray(b_attn, np.float32)
    W_o = np.asarray(W_o, np.float32)
    b_o = np.asarray(b_o, np.float32)
    F8NP = ml_dtypes.float8_e4m3

    B = x.shape[0]
    in_maps = []
    for core in range(8):
        b, hg = divmod(core, 2)
        sl = slice(hg * J, (hg + 1) * J)
        w_l = np.concatenate(
            [W_attn[sl], W_attn[C + hg * J : C + (hg + 1) * J],
             W_attn[2 * C + hg * J : 2 * C + (hg + 1) * J]], axis=0
        )  # [1152, 768]
        b_l = np.concatenate(
            [b_attn[sl], b_attn[C + hg * J : C + (hg + 1) * J],
             b_attn[2 * C + hg * J : 2 * C + (hg + 1) * J]], axis=0
        )  # [1152]
        xt = np.ascontiguousarray(x[b].T)              # [768, 2048]
        wt = np.ascontiguousarray(w_l.T)               # [768, 1152]
        in_maps.append({
            "xt32": np.ascontiguousarray(xt[:, :512]).astype(ml_dtypes.bfloat16),
            "xt8": xt.astype(F8NP),
            "wt32": wt.astype(ml_dtypes.bfloat16),
            "wt8": wt.astype(F8NP),
            "wot": np.ascontiguousarray(W_o[:, sl].T),  # [384, 768]
            "bqk": np.ascontiguousarray(b_l[: 2 * J]),
            "bvf": np.broadcast_to(b_l[2 * J :], (P, J)).copy(),
        })

    nc = _get_nc()
    LAST_RESULTS = bass_utils.run_bass_kernel_spmd(
        nc, in_maps, core_ids=list(range(8)),
        trace=bool(int(os.environ.get("KERNEL_TRACE", "0"))),
    )
    parts = [r["out"] for r in LAST_RESULTS.results]

    out = np.empty((B, T, C), np.float32)
    for b in range(B):
        out[b] = (parts[2 * b] + parts[2 * b + 1]).T + b_o
    return out


# revision 12
# speedup vs baseline: 1.5413x; 1.0002x over previous
"""Multi-head causal self-attention (B=4, T=2048, C=768, H=12) on 8 trn2 cores.

Sharding: core c handles batch b = c//2 and head-group hg = c%2 (6 heads each).
Each core computes its QKV projection slice, causal attention for its 6 heads,
and a partial output projection (768x2048, transposed). Host sums the two
partials per batch, transposes back, and adds b_o. No cross-core collectives.

Key speed structure vs the fp32r baseline:
- all inputs are pre-transposed (and pre-quantized to fp8e4m3 where used as
  fp8) on the HOST, so the kernel does zero on-chip input transposes;
- QKV projection and the PV matmul run as fp8 DoubleRow matmuls (2 k-tiles
  of 128 contracted per pass at 0.5 cycles/row) except where softmax rows
  have too few summands to average out fp8 noise: rows q < 512 (and the
  t < 512 slice of QKV) stay fp32r, keeping rel err ~3e-3;
- V is produced in natural [t, d] layout directly by the projection (no V
  transposes); softmax denominators come from an appended ones column;
- exp on the ACT engine writes fp8 att tiles already in the DoubleRow
  [128, 2, cols] rhs layout; causal masking is applied pre-exp in PSUM by
  gpsimd affine_select with a -1e5 fill;
- the attention stream is ordered qt-major (q-chunk of 512 across all heads)
  so output-projection chunks of earlier qt overlap later attention instead
  of forming a serial tail; QKV chunk tt feeds attention block qt=tt, which
  only needs K/V up to (qt+1)*512 (causality).
"""

import math
import os

import numpy as np
import ml_dtypes

import concourse.bass as bass
from concourse import bacc
import concourse.mybir as mybir
import concourse.tile as tile
from concourse import bass_utils
from concourse.bass import ts

F32 = mybir.dt.float32
F32R = mybir.dt.float32r
F8 = mybir.dt.float8e4
BF16 = mybir.dt.bfloat16
DR = mybir.MatmulPerfMode.DoubleRow

P = 128
T = 2048          # sequence length
C = 768           # embed dim
CS = C // P       # 6 contraction chunks
HL = 6            # heads per core
HD = 64           # head dim
J = HL * HD       # 384 local y-feature dim
JS = J // P       # 3
O = 3 * J         # 1152 rows of the local W_attn slice (q|k|v)
OB = O // P       # 9
QKOB = 6          # q,k row blocks
OUTB = C // P     # 6 output row blocks
NQT = 4           # 512-col q chunks
NPAIR = 8         # 256-row k pairs
HDP = 72          # padded head stride in vaug (dual-fp8 needs 16B-aligned steps)
SCALE = 1.0 / math.sqrt(HD)


def _build_bass():
    nc = bacc.Bacc("TRN2", target_bir_lowering=False, debug=False)
    xt32_d = nc.dram_tensor("xt32", [C, 512], BF16, kind="ExternalInput").ap()
    xt8_d = nc.dram_tensor("xt8", [C, T], F8, kind="ExternalInput").ap()
    wt32_d = nc.dram_tensor("wt32", [C, O], BF16, kind="ExternalInput").ap()
    wt8_d = nc.dram_tensor("wt8", [C, O], F8, kind="ExternalInput").ap()
    wot_d = nc.dram_tensor("wot", [J, C], F32R, kind="ExternalInput").ap()
    bqk_d = nc.dram_tensor("bqk", [2 * J], F32, kind="ExternalInput").ap()
    bvf_d = nc.dram_tensor("bvf", [P, J], F32, kind="ExternalInput").ap()
    out_d = nc.dram_tensor("out", [C, T], F32, kind="ExternalOutput").ap()

    with tile.TileContext(nc) as tc, nc.allow_low_precision(
        reason="fp8 doublerow + fp32r pipeline; fp32 PSUM accumulation"
    ):
        _emit_kernel(tc, xt32_d, xt8_d, wt32_d, wt8_d, wot_d, bqk_d, bvf_d, out_d)
    nc.compile()
    return nc


def _emit_kernel(tc, xt32_d, xt8_d, wt32_d, wt8_d, wot_d, bqk_d, bvf_d, out_d):
    nc = tc.nc

    xt32_r = xt32_d.rearrange("(cs p) t -> p cs t", p=P)   # [128, 6, 512]
    xt8_r = xt8_d.rearrange("(cs p) t -> p cs t", p=P)     # [128, 6, 2048]
    wt32_r = wt32_d.rearrange("(cs p) o -> p cs o", p=P)   # [128, 6, 1152]
    wt8_r = wt8_d.rearrange("(cs p) o -> p cs o", p=P)     # [128, 6, 1152]
    wot_r = wot_d.rearrange("(jb p) o -> p jb o", p=P)     # [128, 3, 768]
    bqk_r = bqk_d.rearrange("(a p) -> p a", p=P)           # [128, 6]
    out_r = out_d.rearrange("(ob p) t -> p ob t", p=P)     # [128, 6, 2048]

    with (
        tc.tile_pool(name="persist", bufs=1) as persist,
        tc.tile_pool(name="att", bufs=4) as attp,
        tc.tile_pool(name="att32", bufs=3) as attp32,
        tc.tile_pool(name="small", bufs=3) as small,
        tc.tile_pool(name="stage", bufs=3) as stage,
        tc.tile_pool(name="oacc", bufs=6) as oaccp,
        tc.tile_pool(name="otail", bufs=1) as otailp,
        tc.tile_pool(name="ps_sp", bufs=2, space="PSUM") as ps_sp,
        tc.tile_pool(name="ps_ya", bufs=2, space="PSUM") as ps_ya,
        tc.tile_pool(name="ps_mm", bufs=2, space="PSUM") as ps_mm,
    ):
        # ---- persistent SBUF tensors
        xt32 = persist.tile([P, CS, 512], BF16)     # 6KB/part
        xt8 = persist.tile([P, CS, T], F8)          # 12KB
        wt32 = persist.tile([P, CS, O], BF16)       # 13.5KB
        wt8 = persist.tile([P, CS, O], F8)          # 6.75KB
        wot = persist.tile([P, JS, C], F32R)        # 9KB
        bqk = persist.tile([P, QKOB], F32)
        bvf = persist.tile([P, J], F32)
        qkT = persist.tile([P, QKOB, T], F32R)      # 48KB  (q ob 0-2, k ob 3-5)
        yT = persist.tile([P, JS, T], F32R)         # 24KB
        vaug8 = persist.tile([P, NPAIR, 2, HL, HDP], F8)      # 6.75KB
        vaug32 = persist.tile([P, 2, 2, HL, HDP], F32R)       # 6.75KB (k<512)
        onesf = small.tile([P, HD], F32, tag="init", name="onesf")
        nc.vector.memset(onesf, 1.0)
        # ones columns of vaug (fp8 1.0 and f32 1.0)
        ones2h = onesf[:, 0 : 2 * HL].rearrange("p (a b) -> p a b", b=HL)
        for pair in range(NPAIR):
            nc.vector.tensor_copy(vaug8[:, pair, :, :, HD], ones2h)
        for pair in range(2):
            nc.vector.tensor_copy(vaug32[:, pair, :, :, HD], ones2h)

        # ---- input DMAs (halved for queue parallelism)
        def dma2(dst, src, axis_len):
            h = axis_len // 2
            nc.sync.dma_start(dst[..., :h], src[..., :h])
            nc.sync.dma_start(dst[..., h:], src[..., h:])

        nc.sync.dma_start(xt32, xt32_r)
        nc.sync.dma_start(wt32[:, :, 0:J], wt32_r[:, :, 0:J])
        nc.sync.dma_start(wt32[:, :, J : 2 * J], wt32_r[:, :, J : 2 * J])
        nc.sync.dma_start(wt32[:, :, 2 * J :], wt32_r[:, :, 2 * J :])
        nc.sync.dma_start(bqk, bqk_r)
        nc.sync.dma_start(bvf, bvf_d)
        dma2(wt8, wt8_r, O)
        dma2(xt8, xt8_r, T)
        dma2(wot, wot_r, C)

        # PE p-state warmup: ~10 small matmuls on the ones tile keep the
        # tensor engine continuously busy through the input-DMA wait so real
        # work starts at full clock instead of mid-ramp.
        pwarm = ps_mm.tile([P, 512], F32, tag="mm", name="pwarm")
        for _ in range(10):
            nc.tensor.matmul(
                pwarm[0:HD, 0:HD], onesf, onesf, start=True, stop=True
            )

        # ================= building blocks =================

        def emit_qkv32(ob):
            # qkT[:, ob, 0:512] for q/k section ob (0..5), fp32r, t < 512
            pq = ps_mm.tile([P, 512], F32, tag="mm")
            for cs in range(CS):
                nc.tensor.matmul(
                    pq, wt32[:, cs, ts(ob, P)], xt32[:, cs, :],
                    start=(cs == 0), stop=(cs == CS - 1),
                )
            nc.vector.tensor_scalar_add(qkT[:, ob, 0:512], pq, bqk[:, ob : ob + 1])

        def emit_qkv8(ob, tt):
            # qkT[:, ob, tt*512:+512] fp8 DoubleRow, tt in 1..3
            pq = ps_mm.tile([P, 512], F32, tag="mm")
            for i in range(3):
                nc.tensor.matmul(
                    pq,
                    wt8[:, 2 * i : 2 * i + 2, ts(ob, P)],
                    xt8[:, 2 * i : 2 * i + 2, ts(tt, 512)],
                    start=(i == 0), stop=(i == 2), perf_mode=DR,
                )
            nc.vector.tensor_scalar_add(
                qkT[:, ob, ts(tt, 512)], pq, bqk[:, ob : ob + 1]
            )

        def emit_v32(tb):
            # natural-layout v for t-block tb (0..3), fp32r -> vaug32 AND vaug8
            pvt = ps_mm.tile([P, 512], F32, tag="mm", name="pvt")
            pv = pvt[:, 0:J]
            for cs in range(CS):
                nc.tensor.matmul(
                    pv, xt32[:, cs, ts(tb, P)], wt32[:, cs, 2 * J : 3 * J],
                    start=(cs == 0), stop=(cs == CS - 1),
                )
            pair, i = divmod(tb, 2)
            dst32 = vaug32[:, pair, i, :, 0:HD]
            dst8 = vaug8[:, pair, i, :, 0:HD]
            nc.vector.tensor_add(dst32, pv.rearrange("p (h d) -> p h d", d=HD),
                                 bvf.rearrange("p (h d) -> p h d", d=HD))
            nc.vector.tensor_add(dst8, pv.rearrange("p (h d) -> p h d", d=HD),
                                 bvf.rearrange("p (h d) -> p h d", d=HD))

        def emit_v8(tb):
            # natural-layout v for t-block tb (4..15), fp8 DoubleRow -> vaug8
            pvt = ps_mm.tile([P, 512], F32, tag="mm", name="pvt")
            pv = pvt[:, 0:J]
            for i in range(3):
                nc.tensor.matmul(
                    pv,
                    xt8[:, 2 * i : 2 * i + 2, ts(tb, P)],
                    wt8[:, 2 * i : 2 * i + 2, 2 * J : 3 * J],
                    start=(i == 0), stop=(i == 2), perf_mode=DR,
                )
            pair, i = divmod(tb, 2)
            nc.vector.tensor_add(
                vaug8[:, pair, i, :, 0:HD],
                pv.rearrange("p (h d) -> p h d", d=HD),
                bvf.rearrange("p (h d) -> p h d", d=HD),
            )

        def emit_outproj(tt):
            # part^T[o, tt*512:+512] = sum_j wot[j, o] yT[j, t]
            for ob in range(OUTB):
                po = ps_mm.tile([P, 512], F32, tag="mm")
                for js in range(JS):
                    nc.tensor.matmul(
                        po, wot[:, js, ts(ob, P)], yT[:, js, ts(tt, 512)],
                        start=(js == 0), stop=(js == JS - 1),
                    )
                osb = stage.tile([P, 512], F32, tag="osb")
                nc.vector.tensor_copy(osb, po)
                nc.sync.dma_start(out_r[:, ob, ts(tt, 512)], osb)

        oacc_tiles = {}

        def emit_outproj_p1(tt, ob):
            # heads 0-3 contribution (jb 0,1) -> SBUF accumulator
            po = ps_mm.tile([P, 512], F32, tag="mm")
            for js in range(JS - 1):
                nc.tensor.matmul(
                    po, wot[:, js, ts(ob, P)], yT[:, js, ts(tt, 512)],
                    start=(js == 0), stop=(js == JS - 2),
                )
            oa = oaccp.tile([P, 512], F32, tag="oacc", name=f"oa{ob}")
            nc.vector.tensor_copy(oa, po)
            oacc_tiles[ob] = oa

        otail = otailp.tile([P, OUTB, 512], F32)

        def emit_outproj_p2(tt, ob):
            # heads 4,5 (jb 2) + accumulator -> batched stage tile
            po = ps_mm.tile([P, 512], F32, tag="mm")
            nc.tensor.matmul(
                po, wot[:, JS - 1, ts(ob, P)], yT[:, JS - 1, ts(tt, 512)],
                start=True, stop=True,
            )
            nc.vector.tensor_add(otail[:, ob, :], po, oacc_tiles[ob])
            if ob == 2:
                nc.sync.dma_start(out_r[:, 0:3, ts(tt, 512)], otail[:, 0:3, :])
            if ob == OUTB - 1:
                nc.sync.dma_start(out_r[:, 3:6, ts(tt, 512)], otail[:, 3:6, :])

        # ================= attention =================
        # unit (h, qt, p): q cols [q0, (qt+1)*512), k pair p (256 rows)

        def unit_geometry(qt, p):
            q0 = max(p * 256, qt * 512)
            cols = (qt + 1) * 512 - q0
            rel = q0 - qt * 512          # 0 or 256
            diag = q0 == p * 256
            return q0, cols, rel, diag

        def emit_scores_exp(h, qt, p):
            """scores (PE) + mask (Pool) + exp (ACT) -> att tile for the unit."""
            q0, cols, rel, diag = unit_geometry(qt, p)
            p0 = (h % 2) * HD
            qTs = qkT[p0 : p0 + HD, h // 2, :]
            kTs = qkT[p0 : p0 + HD, 3 + h // 2, :]
            sp = ps_sp.tile([P, 2, 512], F32, tag="sp")
            # block A (k rows 2p*128..+128): valid from q >= 2p*128 <= q0
            nc.tensor.matmul(
                sp[:, 0, 0:cols], kTs[:, ts(2 * p, P)], qTs[:, q0 : q0 + cols],
                start=True, stop=True,
            )
            # block B: valid from q >= (2p+1)*128; on diagonal units the wedge
            # [0,128) holds finite wrong-side scores, zeroed post-exp below
            nc.tensor.matmul(
                sp[:, 1, 0:cols],
                kTs[:, ts(2 * p + 1, P)], qTs[:, q0 : q0 + cols],
                start=True, stop=True,
            )
            if qt == 0:
                att = attp32.tile([P, 2, 512], F32R, tag="att32")
            else:
                att = attp.tile([P, 2, 512], F8, tag="att")
            nc.scalar.activation(
                att[:, :, 0:cols], sp[:, :, 0:cols],
                mybir.ActivationFunctionType.Exp, scale=SCALE,
            )
            if diag:
                nc.gpsimd.affine_select(
                    out=att[:, :, 0 : 2 * P], in_=att[:, :, 0 : 2 * P],
                    compare_op=mybir.AluOpType.is_ge,
                    fill=0.0, base=0, channel_multiplier=-1,
                    pattern=[[-P, 2], [1, 2 * P]],
                )
            return att

        def emit_pv(h, qt, p, att, ya):
            q0, cols, rel, diag = unit_geometry(qt, p)
            start = p == 0
            stop = p == 2 * qt + 1
            if qt == 0:
                for i in range(2):
                    nc.tensor.matmul(
                        ya[0 : HD + 1, rel : rel + cols],
                        vaug32[:, p, i, h, 0 : HD + 1],
                        att[:, i, 0:cols],
                        start=(start and i == 0), stop=(stop and i == 1),
                    )
            else:
                nc.tensor.matmul(
                    ya[0 : HD + 1, rel : rel + cols],
                    vaug8[:, p, :, h, 0 : HD + 1],
                    att[:, :, 0:cols],
                    start=start, stop=stop, perf_mode=DR,
                )

        norm_q = []

        def flush_norms():
            while norm_q:
                h, qt, ya = norm_q.pop(0)
                p0 = (h % 2) * HD
                rd = small.tile([1, 512], F32R, tag="rd")
                nc.vector.reciprocal(rd, ya[HD : HD + 1, :])
                bcs = small.tile([HD, 512], F32R, tag="bcs")
                nc.gpsimd.partition_broadcast(bcs, rd)
                nc.vector.tensor_mul(
                    out=yT[p0 : p0 + HD, h // 2, ts(qt, 512)],
                    in0=ya[0:HD], in1=bcs,
                )

        # ================= schedule =================
        fillers = []   # (need_qt, fn): must run before attn block need_qt

        def pump(n=1):
            for _ in range(min(n, len(fillers))):
                fillers.pop(0)[1]()

        def drain(up_to_qt):
            while fillers and fillers[0][0] <= up_to_qt:
                fillers.pop(0)[1]()

        # qt0 prerequisites emitted directly (q,k,v for t<512)
        for ob in (0, 3):
            emit_qkv32(ob)
        emit_v32(0)
        emit_v32(1)
        head_ready = [(1, 4), (2, 5)]  # qkv32 obs to emit before heads 2/4

        # fp8 fillers for later qt blocks
        for tt in range(1, NQT):
            for ob in range(QKOB):
                fillers.append((tt, lambda ob=ob, tt=tt: emit_qkv8(ob, tt)))
            for tb in range(4 * tt, 4 * tt + 4):
                fillers.append((tt, lambda tb=tb: emit_v8(tb)))

        pend = []   # deferred PV units: (h, qt, p, att, ya, last)

        def pop_unit():
            h, qt, p, att, ya, last = pend.pop(0)
            emit_pv(h, qt, p, att, ya)
            if last:
                norm_q.append((h, qt, ya))

        n_attn_units = 0
        for qt in range(NQT):
            drain(qt)
            for h in range(HL):
                if qt == 0 and h == 1:
                    emit_v32(2)
                    emit_v32(3)
                if qt == 0 and h in (2, 4):
                    for ob in head_ready[h // 2 - 1]:
                        emit_qkv32(ob)
                if qt == NQT - 1 and h == HL - 1:
                    for ob in range(OUTB):
                        fillers.append((NQT, lambda ob=ob: emit_outproj_p1(3, ob)))
                ya = ps_ya.tile([P, 512], F32, tag="ya", name=f"ya{h}_{qt}")
                for p in range(2 * qt + 2):
                    flush_norms()
                    att = emit_scores_exp(h, qt, p)
                    pend.append((h, qt, p, att, ya, p == 2 * qt + 1))
                    while len(pend) > 2:
                        pop_unit()
                    n_attn_units += 1
                    # don't pull fp8-dependent fillers into the PE stream
                    # before their DMAs have landed (~early qt0)
                    if n_attn_units > 6:
                        pump(1)
            # all heads of qt done -> outproj of this qt becomes available
            while pend:
                pop_unit()
            flush_norms()
            if qt < NQT - 1:
                fillers.append((NQT, lambda tt=qt: emit_outproj(tt)))
        drain(NQT)
        flush_norms()
        for ob in range(OUTB):
            emit_outproj_p2(3, ob)


_NC_CACHE = None
LAST_RESULTS = None


def _get_nc():
    global _NC_CACHE
    if _NC_CACHE is None:
        _NC_CACHE = _build_bass()
    return _NC_CACHE


def kernel(x, W_attn, b_attn, W_o, b_o):
    global LAST_RESULTS
    x = np.asarray(x, np.float32)
    W_attn = np.asarray(W_attn, np.float32)
    b_attn = np.asarray(b_attn, np.float32)
    W_o = np.asarray(W_o, np.float32)
    b_o = np.asarray(b_o, np.float32)
    F8NP = ml_dtypes.float8_e4m3

    B = x.shape[0]
    in_maps = []
    for core in range(8):
        b, hg = divmod(core, 2)
        sl = slice(hg * J, (hg + 1) * J)
        w_l = np.concatenate(
            [W_attn[sl], W_attn[C + hg * J : C + (hg + 1) * J],
             W_attn[2 * C + hg * J : 2 * C + (hg + 1) * J]], axis=0
        )  # [1152, 768]
        b_l = np.concatenate(
            [b_attn[sl], b_attn[C + hg * J : C + (hg + 1) * J],
             b_attn[2 * C + hg * J : 2 * C + (hg + 1) * J]], axis=0
        )  # [1152]
        xt = np.ascontiguousarray(x[b].T)              # [768, 2048]
        wt = np.ascontiguousarray(w_l.T)               # [768, 1152]
        in_maps.append({
            "xt32": np.ascontiguousarray(xt[:, :512]).astype(ml_dtypes.bfloat16),
            "xt8": xt.astype(F8NP),
            "wt32": wt.astype(ml_dtypes.bfloat16),
            "wt8": wt.astype(F8NP),
            "wot": np.ascontiguousarray(W_o[:, sl].T),  # [384, 768]
            "bqk": np.ascontiguousarray(b_l[: 2 * J]),
            "bvf": np.broadcast_to(b_l[2 * J :], (P, J)).copy(),
        })

    nc = _get_nc()
    LAST_RESULTS = bass_utils.run_bass_kernel_spmd(
        nc, in_maps, core_ids=list(range(8)),
        trace=bool(int(os.environ.get("KERNEL_TRACE", "0"))),
    )
    parts = [r["out"] for r in LAST_RESULTS.results]

    out = np.empty((B, T, C), np.float32)
    for b in range(B):
        out[b] = (parts[2 * b] + parts[2 * b + 1]).T + b_o
    return out


# revision 13
# speedup vs baseline: 1.5561x; 1.0096x over previous
"""Multi-head causal self-attention (B=4, T=2048, C=768, H=12) on 8 trn2 cores.

Sharding: core c handles batch b = c//2 and head-group hg = c%2 (6 heads each).
Each core computes its QKV projection slice, causal attention for its 6 heads,
and a partial output projection (768x2048, transposed). Host sums the two
partials per batch, transposes back, and adds b_o. No cross-core collectives.

Key speed structure vs the fp32r baseline:
- all inputs are pre-transposed (and pre-quantized to fp8e4m3 where used as
  fp8) on the HOST, so the kernel does zero on-chip input transposes;
- QKV projection and the PV matmul run as fp8 DoubleRow matmuls (2 k-tiles
  of 128 contracted per pass at 0.5 cycles/row) except where softmax rows
  have too few summands to average out fp8 noise: rows q < 512 (and the
  t < 512 slice of QKV) stay fp32r, keeping rel err ~3e-3;
- V is produced in natural [t, d] layout directly by the projection (no V
  transposes); softmax denominators come from an appended ones column;
- exp on the ACT engine writes fp8 att tiles already in the DoubleRow
  [128, 2, cols] rhs layout; causal masking is applied pre-exp in PSUM by
  gpsimd affine_select with a -1e5 fill;
- the attention stream is ordered qt-major (q-chunk of 512 across all heads)
  so output-projection chunks of earlier qt overlap later attention instead
  of forming a serial tail; QKV chunk tt feeds attention block qt=tt, which
  only needs K/V up to (qt+1)*512 (causality).
"""

import math
import os

import numpy as np
import ml_dtypes

import concourse.bass as bass
from concourse import bacc
import concourse.mybir as mybir
import concourse.tile as tile
from concourse import bass_utils
from concourse.bass import ts

F32 = mybir.dt.float32
F32R = mybir.dt.float32r
F8 = mybir.dt.float8e4
BF16 = mybir.dt.bfloat16
DR = mybir.MatmulPerfMode.DoubleRow

P = 128
T = 2048          # sequence length
C = 768           # embed dim
CS = C // P       # 6 contraction chunks
HL = 6            # heads per core
HD = 64           # head dim
J = HL * HD       # 384 local y-feature dim
JS = J // P       # 3
O = 3 * J         # 1152 rows of the local W_attn slice (q|k|v)
OB = O // P       # 9
QKOB = 6          # q,k row blocks
OUTB = C // P     # 6 output row blocks
NQT = 4           # 512-col q chunks
NPAIR = 8         # 256-row k pairs
HDP = 72          # padded head stride in vaug (dual-fp8 needs 16B-aligned steps)
SCALE = 1.0 / math.sqrt(HD)


def _build_bass():
    nc = bacc.Bacc("TRN2", target_bir_lowering=False, debug=False)
    xt32_d = nc.dram_tensor("xt32", [C, 512], BF16, kind="ExternalInput").ap()
    xt8_d = nc.dram_tensor("xt8", [C, T], F8, kind="ExternalInput").ap()
    wt32_d = nc.dram_tensor("wt32", [C, O], BF16, kind="ExternalInput").ap()
    wt8_d = nc.dram_tensor("wt8", [C, O], F8, kind="ExternalInput").ap()
    wot_d = nc.dram_tensor("wot", [J, C], F32R, kind="ExternalInput").ap()
    bqk_d = nc.dram_tensor("bqk", [2 * J], F32, kind="ExternalInput").ap()
    bvf_d = nc.dram_tensor("bvf", [P, J], F32, kind="ExternalInput").ap()
    out_d = nc.dram_tensor("out", [C, T], F32, kind="ExternalOutput").ap()

    with tile.TileContext(nc) as tc, nc.allow_low_precision(
        reason="fp8 doublerow + fp32r pipeline; fp32 PSUM accumulation"
    ):
        _emit_kernel(tc, xt32_d, xt8_d, wt32_d, wt8_d, wot_d, bqk_d, bvf_d, out_d)
    nc.compile()
    return nc


def _emit_kernel(tc, xt32_d, xt8_d, wt32_d, wt8_d, wot_d, bqk_d, bvf_d, out_d):
    nc = tc.nc

    xt32_r = xt32_d.rearrange("(cs p) t -> p cs t", p=P)   # [128, 6, 512]
    xt8_r = xt8_d.rearrange("(cs p) t -> p cs t", p=P)     # [128, 6, 2048]
    wt32_r = wt32_d.rearrange("(cs p) o -> p cs o", p=P)   # [128, 6, 1152]
    wt8_r = wt8_d.rearrange("(cs p) o -> p cs o", p=P)     # [128, 6, 1152]
    wot_r = wot_d.rearrange("(jb p) o -> p jb o", p=P)     # [128, 3, 768]
    bqk_r = bqk_d.rearrange("(a p) -> p a", p=P)           # [128, 6]
    out_r = out_d.rearrange("(ob p) t -> p ob t", p=P)     # [128, 6, 2048]

    with (
        tc.tile_pool(name="persist", bufs=1) as persist,
        tc.tile_pool(name="att", bufs=4) as attp,
        tc.tile_pool(name="att32", bufs=3) as attp32,
        tc.tile_pool(name="small", bufs=3) as small,
        tc.tile_pool(name="stage", bufs=3) as stage,
        tc.tile_pool(name="oacc", bufs=6) as oaccp,
        tc.tile_pool(name="otail", bufs=1) as otailp,
        tc.tile_pool(name="ps_sp", bufs=2, space="PSUM") as ps_sp,
        tc.tile_pool(name="ps_ya", bufs=2, space="PSUM") as ps_ya,
        tc.tile_pool(name="ps_mm", bufs=2, space="PSUM") as ps_mm,
    ):
        # ---- persistent SBUF tensors
        xt32 = persist.tile([P, CS, 512], BF16)     # 6KB/part
        xt8 = persist.tile([P, CS, T], F8)          # 12KB
        wt32 = persist.tile([P, CS, O], BF16)       # 13.5KB
        wt8 = persist.tile([P, CS, O], F8)          # 6.75KB
        wot = persist.tile([P, JS, C], F32R)        # 9KB
        bqk = persist.tile([P, QKOB], F32)
        bvf = persist.tile([P, J], F32)
        qkT = persist.tile([P, QKOB, T], F32R)      # 48KB  (q ob 0-2, k ob 3-5)
        yT = persist.tile([P, JS, T], F32R)         # 24KB
        vaug8 = persist.tile([P, NPAIR, 2, HL, HDP], F8)      # 6.75KB
        vaug32 = persist.tile([P, 2, 2, HL, HDP], F32R)       # 6.75KB (k<512)
        onesf = small.tile([P, HD], F32, tag="init", name="onesf")
        nc.vector.memset(onesf, 1.0)
        # ones columns of vaug (fp8 1.0 and f32 1.0)
        ones2h = onesf[:, 0 : 2 * HL].rearrange("p (a b) -> p a b", b=HL)
        for pair in range(NPAIR):
            nc.vector.tensor_copy(vaug8[:, pair, :, :, HD], ones2h)
        for pair in range(2):
            nc.vector.tensor_copy(vaug32[:, pair, :, :, HD], ones2h)

        # ---- input DMAs (halved for queue parallelism)
        def dma2(dst, src, axis_len):
            h = axis_len // 2
            nc.sync.dma_start(dst[..., :h], src[..., :h])
            nc.sync.dma_start(dst[..., h:], src[..., h:])

        nc.sync.dma_start(xt32, xt32_r)
        nc.sync.dma_start(wt32[:, :, 0:P], wt32_r[:, :, 0:P])
        nc.sync.dma_start(wt32[:, :, 3 * P : 4 * P], wt32_r[:, :, 3 * P : 4 * P])
        nc.sync.dma_start(wt32[:, :, P : 3 * P], wt32_r[:, :, P : 3 * P])
        nc.sync.dma_start(wt32[:, :, 4 * P : 2 * J], wt32_r[:, :, 4 * P : 2 * J])
        nc.sync.dma_start(wt32[:, :, 2 * J :], wt32_r[:, :, 2 * J :])
        nc.sync.dma_start(bqk, bqk_r)
        nc.sync.dma_start(bvf, bvf_d)
        dma2(wt8, wt8_r, O)
        dma2(xt8, xt8_r, T)
        dma2(wot, wot_r, C)

        # PE p-state warmup: ~10 small matmuls on the ones tile keep the
        # tensor engine continuously busy through the input-DMA wait so real
        # work starts at full clock instead of mid-ramp.
        pwarm = ps_mm.tile([P, 512], F32, tag="mm", name="pwarm")
        for _ in range(40):
            nc.tensor.matmul(
                pwarm[0:HD, 0:HD], onesf, onesf, start=True, stop=True
            )

        # ================= building blocks =================

        def emit_qkv32(ob):
            # qkT[:, ob, 0:512] for q/k section ob (0..5), fp32r, t < 512
            pq = ps_mm.tile([P, 512], F32, tag="mm")
            for cs in range(CS):
                nc.tensor.matmul(
                    pq, wt32[:, cs, ts(ob, P)], xt32[:, cs, :],
                    start=(cs == 0), stop=(cs == CS - 1),
                )
            nc.vector.tensor_scalar_add(qkT[:, ob, 0:512], pq, bqk[:, ob : ob + 1])

        def emit_qkv8(ob, tt):
            # qkT[:, ob, tt*512:+512] fp8 DoubleRow, tt in 1..3
            pq = ps_mm.tile([P, 512], F32, tag="mm")
            for i in range(3):
                nc.tensor.matmul(
                    pq,
                    wt8[:, 2 * i : 2 * i + 2, ts(ob, P)],
                    xt8[:, 2 * i : 2 * i + 2, ts(tt, 512)],
                    start=(i == 0), stop=(i == 2), perf_mode=DR,
                )
            nc.vector.tensor_scalar_add(
                qkT[:, ob, ts(tt, 512)], pq, bqk[:, ob : ob + 1]
            )

        def emit_v32(tb):
            # natural-layout v for t-block tb (0..3), fp32r -> vaug32 AND vaug8
            pvt = ps_mm.tile([P, 512], F32, tag="mm", name="pvt")
            pv = pvt[:, 0:J]
            for cs in range(CS):
                nc.tensor.matmul(
                    pv, xt32[:, cs, ts(tb, P)], wt32[:, cs, 2 * J : 3 * J],
                    start=(cs == 0), stop=(cs == CS - 1),
                )
            pair, i = divmod(tb, 2)
            dst32 = vaug32[:, pair, i, :, 0:HD]
            dst8 = vaug8[:, pair, i, :, 0:HD]
            nc.vector.tensor_add(dst32, pv.rearrange("p (h d) -> p h d", d=HD),
                                 bvf.rearrange("p (h d) -> p h d", d=HD))
            nc.vector.tensor_add(dst8, pv.rearrange("p (h d) -> p h d", d=HD),
                                 bvf.rearrange("p (h d) -> p h d", d=HD))

        def emit_v8(tb):
            # natural-layout v for t-block tb (4..15), fp8 DoubleRow -> vaug8
            pvt = ps_mm.tile([P, 512], F32, tag="mm", name="pvt")
            pv = pvt[:, 0:J]
            for i in range(3):
                nc.tensor.matmul(
                    pv,
                    xt8[:, 2 * i : 2 * i + 2, ts(tb, P)],
                    wt8[:, 2 * i : 2 * i + 2, 2 * J : 3 * J],
                    start=(i == 0), stop=(i == 2), perf_mode=DR,
                )
            pair, i = divmod(tb, 2)
            nc.vector.tensor_add(
                vaug8[:, pair, i, :, 0:HD],
                pv.rearrange("p (h d) -> p h d", d=HD),
                bvf.rearrange("p (h d) -> p h d", d=HD),
            )

        def emit_outproj(tt):
            # part^T[o, tt*512:+512] = sum_j wot[j, o] yT[j, t]
            for ob in range(OUTB):
                po = ps_mm.tile([P, 512], F32, tag="mm")
                for js in range(JS):
                    nc.tensor.matmul(
                        po, wot[:, js, ts(ob, P)], yT[:, js, ts(tt, 512)],
                        start=(js == 0), stop=(js == JS - 1),
                    )
                osb = stage.tile([P, 512], F32, tag="osb")
                nc.vector.tensor_copy(osb, po)
                nc.sync.dma_start(out_r[:, ob, ts(tt, 512)], osb)

        oacc_tiles = {}

        def emit_outproj_p1(tt, ob):
            # heads 0-3 contribution (jb 0,1) -> SBUF accumulator
            po = ps_mm.tile([P, 512], F32, tag="mm")
            for js in range(JS - 1):
                nc.tensor.matmul(
                    po, wot[:, js, ts(ob, P)], yT[:, js, ts(tt, 512)],
                    start=(js == 0), stop=(js == JS - 2),
                )
            oa = oaccp.tile([P, 512], F32, tag="oacc", name=f"oa{ob}")
            nc.vector.tensor_copy(oa, po)
            oacc_tiles[ob] = oa

        otail = otailp.tile([P, OUTB, 512], F32)

        def emit_outproj_p2(tt, ob):
            # heads 4,5 (jb 2) + accumulator -> batched stage tile
            po = ps_mm.tile([P, 512], F32, tag="mm")
            nc.tensor.matmul(
                po, wot[:, JS - 1, ts(ob, P)], yT[:, JS - 1, ts(tt, 512)],
                start=True, stop=True,
            )
            nc.vector.tensor_add(otail[:, ob, :], po, oacc_tiles[ob])
            if ob == 2:
                nc.sync.dma_start(out_r[:, 0:3, ts(tt, 512)], otail[:, 0:3, :])
            if ob == OUTB - 1:
                nc.sync.dma_start(out_r[:, 3:6, ts(tt, 512)], otail[:, 3:6, :])

        # ================= attention =================
        # unit (h, qt, p): q cols [q0, (qt+1)*512), k pair p (256 rows)

        def unit_geometry(qt, p):
            q0 = max(p * 256, qt * 512)
            cols = (qt + 1) * 512 - q0
            rel = q0 - qt * 512          # 0 or 256
            diag = q0 == p * 256
            return q0, cols, rel, diag

        def emit_scores_exp(h, qt, p):
            """scores (PE) + mask (Pool) + exp (ACT) -> att tile for the unit."""
            q0, cols, rel, diag = unit_geometry(qt, p)
            p0 = (h % 2) * HD
            qTs = qkT[p0 : p0 + HD, h // 2, :]
            kTs = qkT[p0 : p0 + HD, 3 + h // 2, :]
            sp = ps_sp.tile([P, 2, 512], F32, tag="sp")
            # block A (k rows 2p*128..+128): valid from q >= 2p*128 <= q0
            nc.tensor.matmul(
                sp[:, 0, 0:cols], kTs[:, ts(2 * p, P)], qTs[:, q0 : q0 + cols],
                start=True, stop=True,
            )
            # block B: valid from q >= (2p+1)*128; on diagonal units the wedge
            # [0,128) holds finite wrong-side scores, zeroed post-exp below
            nc.tensor.matmul(
                sp[:, 1, 0:cols],
                kTs[:, ts(2 * p + 1, P)], qTs[:, q0 : q0 + cols],
                start=True, stop=True,
            )
            if qt == 0:
                att = attp32.tile([P, 2, 512], F32R, tag="att32")
            else:
                att = attp.tile([P, 2, 512], F8, tag="att")
            nc.scalar.activation(
                att[:, :, 0:cols], sp[:, :, 0:cols],
                mybir.ActivationFunctionType.Exp, scale=SCALE,
            )
            if diag:
                nc.gpsimd.affine_select(
                    out=att[:, :, 0 : 2 * P], in_=att[:, :, 0 : 2 * P],
                    compare_op=mybir.AluOpType.is_ge,
                    fill=0.0, base=0, channel_multiplier=-1,
                    pattern=[[-P, 2], [1, 2 * P]],
                )
            return att

        def emit_pv(h, qt, p, att, ya):
            q0, cols, rel, diag = unit_geometry(qt, p)
            start = p == 0
            stop = p == 2 * qt + 1
            if qt == 0:
                for i in range(2):
                    nc.tensor.matmul(
                        ya[0 : HD + 1, rel : rel + cols],
                        vaug32[:, p, i, h, 0 : HD + 1],
                        att[:, i, 0:cols],
                        start=(start and i == 0), stop=(stop and i == 1),
                    )
            else:
                nc.tensor.matmul(
                    ya[0 : HD + 1, rel : rel + cols],
                    vaug8[:, p, :, h, 0 : HD + 1],
                    att[:, :, 0:cols],
                    start=start, stop=stop, perf_mode=DR,
                )

        norm_q = []

        def flush_norms():
            while norm_q:
                h, qt, ya = norm_q.pop(0)
                p0 = (h % 2) * HD
                rd = small.tile([1, 512], F32R, tag="rd")
                nc.vector.reciprocal(rd, ya[HD : HD + 1, :])
                bcs = small.tile([HD, 512], F32R, tag="bcs")
                nc.gpsimd.partition_broadcast(bcs, rd)
                nc.vector.tensor_mul(
                    out=yT[p0 : p0 + HD, h // 2, ts(qt, 512)],
                    in0=ya[0:HD], in1=bcs,
                )

        # ================= schedule =================
        fillers = []   # (need_qt, fn): must run before attn block need_qt

        def pump(n=1):
            for _ in range(min(n, len(fillers))):
                fillers.pop(0)[1]()

        def drain(up_to_qt):
            while fillers and fillers[0][0] <= up_to_qt:
                fillers.pop(0)[1]()

        # qt0 prerequisites emitted directly (q,k,v for t<512)
        for ob in (0, 3):
            emit_qkv32(ob)
        emit_v32(0)
        emit_v32(1)
        head_ready = [(1, 4), (2, 5)]  # qkv32 obs to emit before heads 2/4

        # fp8 fillers for later qt blocks
        for tt in range(1, NQT):
            for ob in range(QKOB):
                fillers.append((tt, lambda ob=ob, tt=tt: emit_qkv8(ob, tt)))
            for tb in range(4 * tt, 4 * tt + 4):
                fillers.append((tt, lambda tb=tb: emit_v8(tb)))

        pend = []   # deferred PV units: (h, qt, p, att, ya, last)

        def pop_unit():
            h, qt, p, att, ya, last = pend.pop(0)
            emit_pv(h, qt, p, att, ya)
            if last:
                norm_q.append((h, qt, ya))

        n_attn_units = 0
        for qt in range(NQT):
            drain(qt)
            for h in range(HL):
                if qt == 0 and h == 1:
                    emit_v32(2)
                    emit_v32(3)
                if qt == 0 and h in (2, 4):
                    for ob in head_ready[h // 2 - 1]:
                        emit_qkv32(ob)
                if qt == NQT - 1 and h == HL - 1:
                    for ob in range(OUTB):
                        fillers.append((NQT, lambda ob=ob: emit_outproj_p1(3, ob)))
                ya = ps_ya.tile([P, 512], F32, tag="ya", name=f"ya{h}_{qt}")
                for p in range(2 * qt + 2):
                    flush_norms()
                    att = emit_scores_exp(h, qt, p)
                    pend.append((h, qt, p, att, ya, p == 2 * qt + 1))
                    while len(pend) > 2:
                        pop_unit()
                    n_attn_units += 1
                    # don't pull fp8-dependent fillers into the PE stream
                    # before their DMAs have landed (~early qt0)
                    if n_attn_units > 6:
                        pump(1)
            # all heads of qt done -> outproj of this qt becomes available
            while pend:
                pop_unit()
            flush_norms()
            if qt < NQT - 1:
                fillers.append((NQT, lambda tt=qt: emit_outproj(tt)))
        drain(NQT)
        flush_norms()
        for ob in range(OUTB):
            emit_outproj_p2(3, ob)


_NC_CACHE = None
LAST_RESULTS = None


def _get_nc():
    global _NC_CACHE
    if _NC_CACHE is None:
        _NC_CACHE = _build_bass()
    return _NC_CACHE


def kernel(x, W_attn, b_attn, W_o, b_o):
    global LAST_RESULTS
    x = np.asarray(x, np.float32)
    W_attn = np.asarray(W_attn, np.float32)
    b_attn = np.asarray(b_attn, np.float32)
    W_o = np.asarray(W_o, np.float32)
    b_o = np.asarray(b_o, np.float32)
    F8NP = ml_dtypes.float8_e4m3

    B = x.shape[0]
    in_maps = []
    for core in range(8):
        b, hg = divmod(core, 2)
        sl = slice(hg * J, (hg + 1) * J)
        w_l = np.concatenate(
            [W_attn[sl], W_attn[C + hg * J : C + (hg + 1) * J],
             W_attn[2 * C + hg * J : 2 * C + (hg + 1) * J]], axis=0
        )  # [1152, 768]
        b_l = np.concatenate(
            [b_attn[sl], b_attn[C + hg * J : C + (hg + 1) * J],
             b_attn[2 * C + hg * J : 2 * C + (hg + 1) * J]], axis=0
        )  # [1152]
        xt = np.ascontiguousarray(x[b].T)              # [768, 2048]
        wt = np.ascontiguousarray(w_l.T)               # [768, 1152]
        in_maps.append({
            "xt32": np.ascontiguousarray(xt[:, :512]).astype(ml_dtypes.bfloat16),
            "xt8": xt.astype(F8NP),
            "wt32": wt.astype(ml_dtypes.bfloat16),
            "wt8": wt.astype(F8NP),
            "wot": np.ascontiguousarray(W_o[:, sl].T),  # [384, 768]
            "bqk": np.ascontiguousarray(b_l[: 2 * J]),
            "bvf": np.broadcast_to(b_l[2 * J :], (P, J)).copy(),
        })

    nc = _get_nc()
    LAST_RESULTS = bass_utils.run_bass_kernel_spmd(
        nc, in_maps, core_ids=list(range(8)),
        trace=bool(int(os.environ.get("KERNEL_TRACE", "0"))),
    )
    parts = [r["out"] for r in LAST_RESULTS.results]

    out = np.empty((B, T, C), np.float32)
    for b in range(B):
        out[b] = (parts[2 * b] + parts[2 * b + 1]).T + b_o
    return out


# revision 14
# speedup vs baseline: 1.5844x; 1.0182x over previous
"""Multi-head causal self-attention (B=4, T=2048, C=768, H=12) on 8 trn2 cores.

Sharding: core c handles batch b = c//2 and head-group hg = c%2 (6 heads each).
Each core computes its QKV projection slice, causal attention for its 6 heads,
and a partial output projection (768x2048, transposed). Host sums the two
partials per batch, transposes back, and adds b_o. No cross-core collectives.

Key speed structure vs the fp32r baseline:
- all inputs are pre-transposed (and pre-quantized to fp8e4m3 where used as
  fp8) on the HOST, so the kernel does zero on-chip input transposes;
- QKV projection and the PV matmul run as fp8 DoubleRow matmuls (2 k-tiles
  of 128 contracted per pass at 0.5 cycles/row) except where softmax rows
  have too few summands to average out fp8 noise: rows q < 512 (and the
  t < 512 slice of QKV) stay fp32r, keeping rel err ~3e-3;
- V is produced in natural [t, d] layout directly by the projection (no V
  transposes); softmax denominators come from an appended ones column;
- exp on the ACT engine writes fp8 att tiles already in the DoubleRow
  [128, 2, cols] rhs layout; causal masking is applied pre-exp in PSUM by
  gpsimd affine_select with a -1e5 fill;
- the attention stream is ordered qt-major (q-chunk of 512 across all heads)
  so output-projection chunks of earlier qt overlap later attention instead
  of forming a serial tail; QKV chunk tt feeds attention block qt=tt, which
  only needs K/V up to (qt+1)*512 (causality).
"""

import math
import os

import numpy as np
import ml_dtypes

import concourse.bass as bass
from concourse import bacc
import concourse.mybir as mybir
import concourse.tile as tile
from concourse import bass_utils
from concourse.bass import ts

F32 = mybir.dt.float32
F32R = mybir.dt.float32r
F8 = mybir.dt.float8e4
BF16 = mybir.dt.bfloat16
DR = mybir.MatmulPerfMode.DoubleRow

P = 128
T = 2048          # sequence length
C = 768           # embed dim
CS = C // P       # 6 contraction chunks
HL = 6            # heads per core
HD = 64           # head dim
J = HL * HD       # 384 local y-feature dim
JS = J // P       # 3
O = 3 * J         # 1152 rows of the local W_attn slice (q|k|v)
OB = O // P       # 9
QKOB = 6          # q,k row blocks
OUTB = C // P     # 6 output row blocks
NQT = 4           # 512-col q chunks
NPAIR = 8         # 256-row k pairs
HDP = 72          # padded head stride in vaug (dual-fp8 needs 16B-aligned steps)
SCALE = 1.0 / math.sqrt(HD)


def _build_bass():
    nc = bacc.Bacc("TRN2", target_bir_lowering=False, debug=False)
    xt32_d = nc.dram_tensor("xt32", [C, 512], BF16, kind="ExternalInput").ap()
    xt8_d = nc.dram_tensor("xt8", [C, T], F8, kind="ExternalInput").ap()
    wt32_d = nc.dram_tensor("wt32", [C, O], BF16, kind="ExternalInput").ap()
    wt8_d = nc.dram_tensor("wt8", [C, O], F8, kind="ExternalInput").ap()
    wot_d = nc.dram_tensor("wot", [J, C], F32R, kind="ExternalInput").ap()
    bqk_d = nc.dram_tensor("bqk", [2 * J], F32, kind="ExternalInput").ap()
    bvf_d = nc.dram_tensor("bvf", [P, J], F32, kind="ExternalInput").ap()
    out_d = nc.dram_tensor("out", [C, T], F32, kind="ExternalOutput").ap()

    with tile.TileContext(nc) as tc, nc.allow_low_precision(
        reason="fp8 doublerow + fp32r pipeline; fp32 PSUM accumulation"
    ):
        _emit_kernel(tc, xt32_d, xt8_d, wt32_d, wt8_d, wot_d, bqk_d, bvf_d, out_d)
    nc.compile()
    return nc


def _emit_kernel(tc, xt32_d, xt8_d, wt32_d, wt8_d, wot_d, bqk_d, bvf_d, out_d):
    nc = tc.nc

    xt32_r = xt32_d.rearrange("(cs p) t -> p cs t", p=P)   # [128, 6, 512]
    xt8_r = xt8_d.rearrange("(cs p) t -> p cs t", p=P)     # [128, 6, 2048]
    wt32_r = wt32_d.rearrange("(cs p) o -> p cs o", p=P)   # [128, 6, 1152]
    wt8_r = wt8_d.rearrange("(cs p) o -> p cs o", p=P)     # [128, 6, 1152]
    wot_r = wot_d.rearrange("(jb p) o -> p jb o", p=P)     # [128, 3, 768]
    bqk_r = bqk_d.rearrange("(a p) -> p a", p=P)           # [128, 6]
    out_r = out_d.rearrange("(ob p) t -> p ob t", p=P)     # [128, 6, 2048]

    with (
        tc.tile_pool(name="persist", bufs=1) as persist,
        tc.tile_pool(name="att", bufs=4) as attp,
        tc.tile_pool(name="att32", bufs=3) as attp32,
        tc.tile_pool(name="small", bufs=3) as small,
        tc.tile_pool(name="stage", bufs=3) as stage,
        tc.tile_pool(name="oacc", bufs=6) as oaccp,
        tc.tile_pool(name="otail", bufs=1) as otailp,
        tc.tile_pool(name="ps_sp", bufs=2, space="PSUM") as ps_sp,
        tc.tile_pool(name="ps_ya", bufs=2, space="PSUM") as ps_ya,
        tc.tile_pool(name="ps_mm", bufs=2, space="PSUM") as ps_mm,
    ):
        # ---- persistent SBUF tensors
        xt32 = persist.tile([P, CS, 512], BF16)     # 6KB/part
        xt8 = persist.tile([P, CS, T], F8)          # 12KB
        wt32 = persist.tile([P, CS, O], BF16)       # 13.5KB
        wt8 = persist.tile([P, CS, O], F8)          # 6.75KB
        wot = persist.tile([P, JS, C], F32R)        # 9KB
        bqk = persist.tile([P, QKOB], F32)
        bvf = persist.tile([P, J], F32)
        qkT = persist.tile([P, QKOB, T], F32R)      # 48KB  (q ob 0-2, k ob 3-5)
        yT = persist.tile([P, JS, T], F32R)         # 24KB
        vaug8 = persist.tile([P, NPAIR, 2, HL, HDP], F8)      # 6.75KB
        vaug32 = persist.tile([P, 2, 2, HL, HDP], F32R)       # 6.75KB (k<512)
        onesf = small.tile([P, HD], F32, tag="init", name="onesf")
        nc.vector.memset(onesf, 1.0)
        # ones columns of vaug (fp8 1.0 and f32 1.0)
        ones2h = onesf[:, 0 : 2 * HL].rearrange("p (a b) -> p a b", b=HL)
        for pair in range(NPAIR):
            nc.vector.tensor_copy(vaug8[:, pair, :, :, HD], ones2h)
        for pair in range(2):
            nc.vector.tensor_copy(vaug32[:, pair, :, :, HD], ones2h)

        # ---- input DMAs (halved for queue parallelism)
        def dma2(dst, src, axis_len):
            h = axis_len // 2
            nc.sync.dma_start(dst[..., :h], src[..., :h])
            nc.sync.dma_start(dst[..., h:], src[..., h:])

        nc.sync.dma_start(xt32, xt32_r)
        nc.sync.dma_start(wt32[:, :, 0:P], wt32_r[:, :, 0:P])
        nc.sync.dma_start(wt32[:, :, 3 * P : 4 * P], wt32_r[:, :, 3 * P : 4 * P])
        nc.sync.dma_start(wt32[:, :, P : 3 * P], wt32_r[:, :, P : 3 * P])
        nc.sync.dma_start(wt32[:, :, 4 * P : 2 * J], wt32_r[:, :, 4 * P : 2 * J])
        nc.sync.dma_start(wt32[:, :, 2 * J :], wt32_r[:, :, 2 * J :])
        nc.sync.dma_start(bqk, bqk_r)
        nc.sync.dma_start(bvf, bvf_d)
        dma2(wt8, wt8_r, O)
        dma2(xt8, xt8_r, T)
        dma2(wot, wot_r, C)

        # PE p-state warmup: ~10 small matmuls on the ones tile keep the
        # tensor engine continuously busy through the input-DMA wait so real
        # work starts at full clock instead of mid-ramp.
        pwarm = ps_mm.tile([P, 512], F32, tag="mm", name="pwarm")
        for _ in range(40):
            nc.tensor.matmul(
                pwarm[0:HD, 0:HD], onesf, onesf, start=True, stop=True
            )

        # ================= building blocks =================

        def emit_qkv32(ob):
            # qkT[:, ob, 0:512] for q/k section ob (0..5), fp32r, t < 512
            pq = ps_mm.tile([P, 512], F32, tag="mm")
            for cs in range(CS):
                nc.tensor.matmul(
                    pq, wt32[:, cs, ts(ob, P)], xt32[:, cs, :],
                    start=(cs == 0), stop=(cs == CS - 1),
                )
            nc.vector.tensor_scalar_add(qkT[:, ob, 0:512], pq, bqk[:, ob : ob + 1])

        def emit_qkv8(ob, tt):
            # qkT[:, ob, tt*512:+512] fp8 DoubleRow, tt in 1..3
            pq = ps_mm.tile([P, 512], F32, tag="mm")
            for i in range(3):
                nc.tensor.matmul(
                    pq,
                    wt8[:, 2 * i : 2 * i + 2, ts(ob, P)],
                    xt8[:, 2 * i : 2 * i + 2, ts(tt, 512)],
                    start=(i == 0), stop=(i == 2), perf_mode=DR,
                )
            nc.vector.tensor_scalar_add(
                qkT[:, ob, ts(tt, 512)], pq, bqk[:, ob : ob + 1]
            )

        def emit_v32(tb):
            # natural-layout v for t-block tb (0..3), fp32r -> vaug32 AND vaug8
            pvt = ps_mm.tile([P, 512], F32, tag="mm", name="pvt")
            pv = pvt[:, 0:J]
            for cs in range(CS):
                nc.tensor.matmul(
                    pv, xt32[:, cs, ts(tb, P)], wt32[:, cs, 2 * J : 3 * J],
                    start=(cs == 0), stop=(cs == CS - 1),
                )
            pair, i = divmod(tb, 2)
            dst32 = vaug32[:, pair, i, :, 0:HD]
            dst8 = vaug8[:, pair, i, :, 0:HD]
            nc.vector.tensor_add(dst32, pv.rearrange("p (h d) -> p h d", d=HD),
                                 bvf.rearrange("p (h d) -> p h d", d=HD))
            nc.vector.tensor_add(dst8, pv.rearrange("p (h d) -> p h d", d=HD),
                                 bvf.rearrange("p (h d) -> p h d", d=HD))

        def emit_v8(tb):
            # natural-layout v for t-block tb (4..15), fp8 DoubleRow -> vaug8
            pvt = ps_mm.tile([P, 512], F32, tag="mm", name="pvt")
            pv = pvt[:, 0:J]
            for i in range(3):
                nc.tensor.matmul(
                    pv,
                    xt8[:, 2 * i : 2 * i + 2, ts(tb, P)],
                    wt8[:, 2 * i : 2 * i + 2, 2 * J : 3 * J],
                    start=(i == 0), stop=(i == 2), perf_mode=DR,
                )
            pair, i = divmod(tb, 2)
            nc.vector.tensor_add(
                vaug8[:, pair, i, :, 0:HD],
                pv.rearrange("p (h d) -> p h d", d=HD),
                bvf.rearrange("p (h d) -> p h d", d=HD),
            )

        def emit_outproj(tt):
            # part^T[o, tt*512:+512] = sum_j wot[j, o] yT[j, t]
            for ob in range(OUTB):
                po = ps_mm.tile([P, 512], F32, tag="mm")
                for js in range(JS):
                    nc.tensor.matmul(
                        po, wot[:, js, ts(ob, P)], yT[:, js, ts(tt, 512)],
                        start=(js == 0), stop=(js == JS - 1),
                    )
                osb = stage.tile([P, 512], F32, tag="osb")
                nc.vector.tensor_copy(osb, po)
                nc.sync.dma_start(out_r[:, ob, ts(tt, 512)], osb)

        oacc_tiles = {}

        def emit_outproj_p1(tt, ob):
            # heads 0-3 contribution (jb 0,1) -> SBUF accumulator
            po = ps_mm.tile([P, 512], F32, tag="mm")
            for js in range(JS - 1):
                nc.tensor.matmul(
                    po, wot[:, js, ts(ob, P)], yT[:, js, ts(tt, 512)],
                    start=(js == 0), stop=(js == JS - 2),
                )
            oa = oaccp.tile([P, 512], F32, tag="oacc", name=f"oa{ob}")
            nc.vector.tensor_copy(oa, po)
            oacc_tiles[ob] = oa

        otail = otailp.tile([P, OUTB, 512], F32)

        def emit_outproj_p2(tt, ob):
            # heads 4,5 (jb 2) + accumulator -> batched stage tile
            po = ps_mm.tile([P, 512], F32, tag="mm")
            nc.tensor.matmul(
                po, wot[:, JS - 1, ts(ob, P)], yT[:, JS - 1, ts(tt, 512)],
                start=True, stop=True,
            )
            nc.vector.tensor_add(otail[:, ob, :], po, oacc_tiles[ob])
            nc.sync.dma_start(out_r[:, ob, ts(tt, 512)], otail[:, ob, :])

        # ================= attention =================
        # unit (h, qt, p): q cols [q0, (qt+1)*512), k pair p (256 rows)

        def unit_geometry(qt, p):
            q0 = max(p * 256, qt * 512)
            cols = (qt + 1) * 512 - q0
            rel = q0 - qt * 512          # 0 or 256
            diag = q0 == p * 256
            return q0, cols, rel, diag

        def emit_scores_exp(h, qt, p):
            """scores (PE) + mask (Pool) + exp (ACT) -> att tile for the unit."""
            q0, cols, rel, diag = unit_geometry(qt, p)
            p0 = (h % 2) * HD
            qTs = qkT[p0 : p0 + HD, h // 2, :]
            kTs = qkT[p0 : p0 + HD, 3 + h // 2, :]
            sp = ps_sp.tile([P, 2, 512], F32, tag="sp")
            # block A (k rows 2p*128..+128): valid from q >= 2p*128 <= q0
            nc.tensor.matmul(
                sp[:, 0, 0:cols], kTs[:, ts(2 * p, P)], qTs[:, q0 : q0 + cols],
                start=True, stop=True,
            )
            # block B: valid from q >= (2p+1)*128; on diagonal units the wedge
            # [0,128) holds finite wrong-side scores, zeroed post-exp below
            nc.tensor.matmul(
                sp[:, 1, 0:cols],
                kTs[:, ts(2 * p + 1, P)], qTs[:, q0 : q0 + cols],
                start=True, stop=True,
            )
            if qt == 0:
                att = attp32.tile([P, 2, 512], F32R, tag="att32")
            else:
                att = attp.tile([P, 2, 512], F8, tag="att")
            nc.scalar.activation(
                att[:, :, 0:cols], sp[:, :, 0:cols],
                mybir.ActivationFunctionType.Exp, scale=SCALE,
            )
            if diag:
                nc.gpsimd.affine_select(
                    out=att[:, :, 0 : 2 * P], in_=att[:, :, 0 : 2 * P],
                    compare_op=mybir.AluOpType.is_ge,
                    fill=0.0, base=0, channel_multiplier=-1,
                    pattern=[[-P, 2], [1, 2 * P]],
                )
            return att

        def emit_pv(h, qt, p, att, ya):
            q0, cols, rel, diag = unit_geometry(qt, p)
            start = p == 2 * qt + 1
            stop = p == 0
            if qt == 0:
                for i in range(2):
                    nc.tensor.matmul(
                        ya[0 : HD + 1, rel : rel + cols],
                        vaug32[:, p, i, h, 0 : HD + 1],
                        att[:, i, 0:cols],
                        start=(start and i == 0), stop=(stop and i == 1),
                    )
            else:
                nc.tensor.matmul(
                    ya[0 : HD + 1, rel : rel + cols],
                    vaug8[:, p, :, h, 0 : HD + 1],
                    att[:, :, 0:cols],
                    start=start, stop=stop, perf_mode=DR,
                )

        norm_q = []

        def flush_norms():
            while norm_q:
                h, qt, ya = norm_q.pop(0)
                p0 = (h % 2) * HD
                rd = small.tile([1, 512], F32R, tag="rd")
                nc.vector.reciprocal(rd, ya[HD : HD + 1, :])
                bcs = small.tile([HD, 512], F32R, tag="bcs")
                nc.gpsimd.partition_broadcast(bcs, rd)
                nc.vector.tensor_mul(
                    out=yT[p0 : p0 + HD, h // 2, ts(qt, 512)],
                    in0=ya[0:HD], in1=bcs,
                )

        # ================= schedule =================
        fillers = []   # (need_qt, fn): must run before attn block need_qt

        def pump(n=1):
            for _ in range(min(n, len(fillers))):
                fillers.pop(0)[1]()

        def drain(up_to_qt):
            while fillers and fillers[0][0] <= up_to_qt:
                fillers.pop(0)[1]()

        # qt0 prerequisites emitted directly (q,k,v for t<512)
        for ob in (0, 3):
            emit_qkv32(ob)
        emit_v32(0)
        emit_v32(1)
        head_ready = [(1, 4), (2, 5)]  # qkv32 obs to emit before heads 2/4

        # fp8 fillers for later qt blocks
        for tt in range(1, NQT):
            for ob in range(QKOB):
                fillers.append((tt, lambda ob=ob, tt=tt: emit_qkv8(ob, tt)))
            for tb in range(4 * tt, 4 * tt + 4):
                fillers.append((tt, lambda tb=tb: emit_v8(tb)))

        pend = []   # deferred PV units: (h, qt, p, att, ya, last)

        def pop_unit():
            h, qt, p, att, ya, last = pend.pop(0)
            emit_pv(h, qt, p, att, ya)
            if last:
                norm_q.append((h, qt, ya))

        n_attn_units = 0
        for qt in range(NQT):
            drain(qt)
            for h in range(HL):
                if qt == 0 and h == 1:
                    emit_v32(2)
                    emit_v32(3)
                if qt == 0 and h in (2, 4):
                    for ob in head_ready[h // 2 - 1]:
                        emit_qkv32(ob)
                if qt == NQT - 1 and h == HL - 1:
                    for ob in range(OUTB):
                        fillers.append((NQT, lambda ob=ob: emit_outproj_p1(3, ob)))
                ya = ps_ya.tile([P, 512], F32, tag="ya", name=f"ya{h}_{qt}")
                for p in reversed(range(2 * qt + 2)):
                    flush_norms()
                    att = emit_scores_exp(h, qt, p)
                    pend.append((h, qt, p, att, ya, p == 0))
                    while len(pend) > 2:
                        pop_unit()
                    n_attn_units += 1
                    # don't pull fp8-dependent fillers into the PE stream
                    # before their DMAs have landed (~early qt0)
                    if n_attn_units > 6:
                        pump(1)
            # all heads of qt done -> outproj of this qt becomes available
            while pend:
                pop_unit()
            flush_norms()
            if qt < NQT - 1:
                fillers.append((NQT, lambda tt=qt: emit_outproj(tt)))
        drain(NQT)
        flush_norms()
        for ob in range(OUTB):
            emit_outproj_p2(3, ob)


_NC_CACHE = None
LAST_RESULTS = None


def _get_nc():
    global _NC_CACHE
    if _NC_CACHE is None:
        _NC_CACHE = _build_bass()
    return _NC_CACHE


def kernel(x, W_attn, b_attn, W_o, b_o):
    global LAST_RESULTS
    x = np.asarray(x, np.float32)
    W_attn = np.asarray(W_attn, np.float32)
    b_attn = np.asarray(b_attn, np.float32)
    W_o = np.asarray(W_o, np.float32)
    b_o = np.asarray(b_o, np.float32)
    F8NP = ml_dtypes.float8_e4m3

    B = x.shape[0]
    in_maps = []
    for core in range(8):
        b, hg = divmod(core, 2)
        sl = slice(hg * J, (hg + 1) * J)
        w_l = np.concatenate(
            [W_attn[sl], W_attn[C + hg * J : C + (hg + 1) * J],
             W_attn[2 * C + hg * J : 2 * C + (hg + 1) * J]], axis=0
        )  # [1152, 768]
        b_l = np.concatenate(
            [b_attn[sl], b_attn[C + hg * J : C + (hg + 1) * J],
             b_attn[2 * C + hg * J : 2 * C + (hg + 1) * J]], axis=0
        )  # [1152]
        xt = np.ascontiguousarray(x[b].T)              # [768, 2048]
        wt = np.ascontiguousarray(w_l.T)               # [768, 1152]
        in_maps.append({
            "xt32": np.ascontiguousarray(xt[:, :512]).astype(ml_dtypes.bfloat16),
            "xt8": xt.astype(F8NP),
            "wt32": wt.astype(ml_dtypes.bfloat16),
            "wt8": wt.astype(F8NP),
            "wot": np.ascontiguousarray(W_o[:, sl].T),  # [384, 768]
            "bqk": np.ascontiguousarray(b_l[: 2 * J]),
            "bvf": np.broadcast_to(b_l[2 * J :], (P, J)).copy(),
        })

    nc = _get_nc()
    LAST_RESULTS = bass_utils.run_bass_kernel_spmd(
        nc, in_maps, core_ids=list(range(8)),
        trace=bool(int(os.environ.get("KERNEL_TRACE", "0"))),
    )
    parts = [r["out"] for r in LAST_RESULTS.results]

    out = np.empty((B, T, C), np.float32)
    for b in range(B):
        out[b] = (parts[2 * b] + parts[2 * b + 1]).T + b_o
    return out
